# revision 4
# baseline (speedup 1.0000x reference)
"""CrossMamba Trainium2 kernel.

Sharding: 8 cores = 4 batches x 2 scan directions. Core b and core 4+b
form a pair that works on batch b; both run the same SPMD program and
differ only in a 4-byte selector in the meta row (sel=1 fwd, 0 bwd).

Wall-clock is dominated by the axon tunnel (~30-100 MB/s shared, ~80 ms
fixed latency per direction; device exec is only ~3.6 ms), so the I/O
contract is tuned for wire bytes and host (single-CPU) cost:
  - all weights are baked into the NEFF as inline Const tensors
    (transferred once at executable load, never per call)
  - activations upload quantized: ctx as an 8-bit code (its noise is
    averaged down by the 768-wide c_in GEMM), q as a 9-bit code
    (low byte + bit-packed high bit). 0.94 MB per core, 7.5 MB total.
    Codes are packed host-side in natural [time, feature] layout with
    contiguous numpy ops only; the device unpacks (int shift/and ops)
    and PE-transposes to feature-major. The +2^(b-1) code bias is
    folded into effective seg biases (for ctx via colsum(c_in_w)), so
    dequantization costs no extra per-element work
  - each core uploads only HALF of its batch's sequence; an on-device
    pair AllGather (over f32-viewed byte buffers) reassembles it
  - per-core shards are device_put as soon as they are packed (puts
    are async), overlapping host packing with the upload stream; the
    output is fetched without an intermediate block_until_ready, which
    hides the execute round trip
  - the backward direction is derived on device: exact 0/1 sel-blends
    choose operand placement, and negative-stride (reversed-AP) copies
    time-flip the data, so fwd and bwd cores run one program
  - out_proj runs operand-swapped so the output is TIME-major, the
    fwd+bwd results are summed with a pair AllReduce, and each core
    downloads its dm-half int8-quantized with a per-time-row f32 scale
    (scale carries the 0.5 fwd/bwd average): 0.53 MB per core, 4.3 MB
    total. Host dequant is two contiguous broadcasts per batch
  - the jitted executable is cached at module level keyed on a weight
    fingerprint; repeat calls skip re-trace/re-compile

Per-core program:
  A0) bounce upload to DRAM, pair AllGather -> full time range
  A) unpack codes, PE-transpose, x = blend(c_in(ctx)+segc', q+segq')
     with sel-driven placement/flip
  B) in_proj (u half) -> causal depthwise conv -> silu -> x_proj acc
  C) in_proj (z half) -> silu -> spill
  D) x_proj epilogue (dt / B / C rows)
  E) dt_proj -> softplus -> delta, dg = delta*u
  F) selective scan: per (channel-block, state): dA = exp(A_s*delta),
     dgB, hardware tensor_tensor_scan, C-readout, state accumulation;
     two passes of 8 states
  G) gate with silu(z), sel-chosen/flipped time half -> time-major
     out_proj, pair AllReduce, int8 quantize, output the dm half

GEMMs run in fp16 (f32 PSUM accumulate), scan math in f32/bf16.
End-to-end relative error vs the fp32 reference: ~9e-3 (gate: 2e-2).
"""
import hashlib
import numpy as np

B, Lq, Lc = 4, 1024, 1024
DQ, DC, DM = 1024, 768, 1024
DS, DCONV = 16, 4
DI, DTR = 2048, 64
L = Lc + Lq              # 2048
NCORE = 8
NE = DI // 128           # 16 u (or z) channel blocks
NK = DM // 128           # 8 k blocks for in_proj
NT = L // 512            # 4 time blocks of 512
NA = (DC + DM) // 128    # 14 row blocks in the packed activation half

_RUN = None              # cached (runner, weight fingerprint)
_WFP = None

_WKEYS = ("c_in_w", "seg_context", "seg_query", "in_proj_w", "conv_w",
          "conv_b", "x_proj_w", "dt_proj_w", "dt_proj_b", "A_log", "D",
          "mamba_out_w")


def _fingerprint(inputs):
    h = hashlib.blake2b(digest_size=16)
    for k in _WKEYS:
        a = np.ascontiguousarray(np.asarray(inputs[k]))
        h.update(k.encode())
        h.update(str(a.shape).encode())
        b = a.view(np.uint8).reshape(-1)
        step = max(1, b.size // 65536)
        h.update(bytes(b[::step][:65536]))
    return h.digest()


def _prep_weights(inputs):
    f32, f16 = np.float32, np.float16
    c_in_w = np.asarray(inputs["c_in_w"], f32)
    segc = np.asarray(inputs["seg_context"], f32).reshape(DM)
    segq = np.asarray(inputs["seg_query"], f32).reshape(DM)
    in_proj_w = np.asarray(inputs["in_proj_w"], f32)
    conv_w = np.asarray(inputs["conv_w"], f32)
    conv_b = np.asarray(inputs["conv_b"], f32)
    x_proj_w = np.asarray(inputs["x_proj_w"], f32)
    dt_proj_w = np.asarray(inputs["dt_proj_w"], f32)
    dt_proj_b = np.asarray(inputs["dt_proj_b"], f32)
    A = (-np.exp(np.asarray(inputs["A_log"], f32))).astype(f32)
    D = np.asarray(inputs["D"], f32)
    out_w = np.asarray(inputs["mamba_out_w"], f32)

    def blk(a, p=128):
        # [n*p, m] -> [p, n*m] with n-major free layout
        n = a.shape[0] // p
        return np.ascontiguousarray(
            a.reshape(n, p, -1).transpose(1, 0, 2).reshape(p, -1))

    return dict(
        Wc=blk(c_in_w.T).astype(f16),                     # [128, 6*1024]
        segc=np.ascontiguousarray(segc.reshape(NK, 128).T),   # [128, 8]
        segq=np.ascontiguousarray(segq.reshape(NK, 128).T),
        # row sums of c_in_w (over the DC axis): used to fold the
        # +512 bias of the 10-bit activation code out of the c_in GEMM
        csum=np.ascontiguousarray(
            c_in_w.sum(axis=1).reshape(NK, 128).T),       # [128, 8]
        Win=np.ascontiguousarray(
            in_proj_w.reshape(32, 128, NK, 128).transpose(0, 3, 2, 1)
            .reshape(32, 128, NK * 128)).astype(f16),     # [32,128,1024]
        Wxp=blk(x_proj_w.T).astype(f16),                  # [128, 16*96]
        Wdt=np.ascontiguousarray(dt_proj_w.T).astype(f16),  # [64, 2048]
        Wout=np.ascontiguousarray(
            out_w.reshape(8, 128, NE, 128).transpose(3, 2, 0, 1)
            .reshape(128, NE * DM)).astype(f16),          # [128, 16*1024]
        convw=blk(conv_w),                                # [128, 16*4]
        convb=conv_b.reshape(NE, 128).T.copy(),
        dtb=dt_proj_b.reshape(NE, 128).T.copy(),
        Ah=blk(A),                                        # [128, 16*16]
        Dh=D.reshape(NE, 128).T.copy(),
    )


def _build(w):
    import concourse.bacc as bacc
    import concourse.tile as tile
    from concourse import mybir

    f32 = mybir.dt.float32
    f16 = mybir.dt.float16
    bf16 = mybir.dt.bfloat16
    u8 = mybir.dt.uint8
    i8 = mybir.dt.int8
    i32 = mybir.dt.int32
    MUL = mybir.AluOpType.mult
    ADD = mybir.AluOpType.add
    SUB = mybir.AluOpType.subtract
    BYP = mybir.AluOpType.bypass
    MAXO = mybir.AluOpType.max
    SHR = mybir.AluOpType.logical_shift_right
    AND = mybir.AluOpType.bitwise_and
    AF = mybir.ActivationFunctionType
    AX = mybir.AxisListType
    PAIRS = [[0, 4], [1, 5], [2, 6], [3, 7]]

    nc = bacc.Bacc("TRN2", target_bir_lowering=False, debug=False,
                   num_devices=NCORE)

    # ---- per-core external inputs ----
    # Quantized activations for this core's half of the batch's [ctx, q]
    # feature-concat, NATURAL [time, feature] layout (host packs with
    # contiguous ops only; the device transposes). Core b carries times
    # 0:512, core 4+b times 512:1024.
    # ctx uses an 8-bit code u = round(x*s_c)+128 (its quantization
    # noise is averaged down by the 768-wide c_in GEMM); q uses a 9-bit
    # code u = round(x*s_q)+256 (it enters x directly).
    #   alo: ctx code bytes (cols 0:768) | q low bytes (cols 768:1792)
    #                                          [512, NA*128] u8
    #   ahi: rows 0:512 pack the q high bit of 8 consecutive features
    #        per byte (little bit order); row 512 carries 16 meta bytes
    #        = f32 [sel, 1/s_ctx, 1/s_q, 0]    [513, 128] u8
    NF = NA * 128            # 1792 features (ctx 768 | q 1024)
    alo_d = nc.dram_tensor("alo", [512, NF], u8, kind="ExternalInput")
    ahi_d = nc.dram_tensor("ahi", [513, 128], u8, kind="ExternalInput")

    # ---- weights baked into the NEFF (loaded once, not per call) ----
    Wc_d = nc.inline_tensor(w["Wc"], name="Wc_i")
    segc_d = nc.inline_tensor(w["segc"], name="segc_i")
    segq_d = nc.inline_tensor(w["segq"], name="segq_i")
    csum_d = nc.inline_tensor(w["csum"], name="csum_i")
    Win_d = nc.inline_tensor(w["Win"], name="Win_i")
    Wxp_d = nc.inline_tensor(w["Wxp"], name="Wxp_i")
    Wdt_d = nc.inline_tensor(w["Wdt"], name="Wdt_i")
    Wout_d = nc.inline_tensor(w["Wout"], name="Wout_i")
    convw_d = nc.inline_tensor(w["convw"], name="convw_i")
    convb_d = nc.inline_tensor(w["convb"], name="convb_i")
    dtb_d = nc.inline_tensor(w["dtb"], name="dtb_i")
    Ah_d = nc.inline_tensor(w["Ah"], name="Ah_i")
    Dh_d = nc.inline_tensor(w["Dh"], name="Dh_i")
    eye_d = nc.inline_tensor(np.eye(128, dtype=np.float16), name="eye_i")

    # ---- DRAM scratch ----
    # bounce/gather buffers are declared f32 (collective-safe dtype) and
    # byte-addressed via bitcast; widths are bytes/4
    NF = NA * 128
    lo_bnc = nc.dram_tensor("lo_bnc", [512, NF // 4], f32)
    hi_bnc = nc.dram_tensor("hi_bnc", [512, 32], f32)
    ag_lo = nc.dram_tensor("ag_lo", [2, 512, NF // 4], f32)
    ag_hi = nc.dram_tensor("ag_hi", [2, 512, 32], f32)
    u_sp = nc.dram_tensor("u_sp", [DI, L], f16)
    zs_sp = nc.dram_tensor("zs_sp", [DI, L], bf16)
    dl_sp = nc.dram_tensor("dl_sp", [DI, L], f16)
    dg_sp = nc.dram_tensor("dg_sp", [DI, L], f16)
    bc_sp = nc.dram_tensor("bc_sp", [2 * DS, L], bf16)
    yacc_sp = nc.dram_tensor("yacc_sp", [DI, L], f32)
    yg_sp = nc.dram_tensor("yg_sp", [DI, L], f16)
    og_sp = nc.dram_tensor("og_sp", [Lq, DM], f16)
    og_sum = nc.dram_tensor("og_sum", [Lq, DM], f16)

    # time-major int8 output + per-time-row f32 scale in cols 512:516
    out_d = nc.dram_tensor("out", [Lq, DM // 2 + 4], u8,
                           kind="ExternalOutput")

    with tile.TileContext(nc) as tc:
        with (
            tc.tile_pool(name="wp", bufs=1) as wp,
            tc.tile_pool(name="ps", bufs=3, space="PSUM") as ps,
        ):
            # ---------- phase A0: bounce + pair AllGather ----------
            with tc.tile_pool(name="p0", bufs=2) as p0:
                for rb in range(4):
                    r0, r1 = rb * 128, (rb + 1) * 128
                    bt = p0.tile([128, NF], u8, tag="bnc")
                    nc.sync.dma_start(bt[:], alo_d[r0:r1, :])
                    nc.sync.dma_start(lo_bnc[r0:r1, :].bitcast(u8), bt[:])
                    ht = p0.tile([128, 128], u8, tag="bnch")
                    nc.sync.dma_start(ht[:], ahi_d[r0:r1, :])
                    nc.sync.dma_start(hi_bnc[r0:r1, :].bitcast(u8), ht[:])
            nc.gpsimd.collective_compute(
                "AllGather", BYP, replica_groups=PAIRS,
                ins=[lo_bnc[:].opt()], outs=[ag_lo[:].opt()])
            nc.gpsimd.collective_compute(
                "AllGather", BYP, replica_groups=PAIRS,
                ins=[hi_bnc[:].opt()], outs=[ag_hi[:].opt()])

            # ---------- small persistent weights ----------
            convw = wp.tile([128, NE * DCONV], f32, tag="convw")
            nc.sync.dma_start(convw[:], convw_d[:])
            convb = wp.tile([128, NE], f32, tag="convb")
            nc.sync.dma_start(convb[:], convb_d[:])
            dtb = wp.tile([128, NE], f32, tag="dtb")
            nc.sync.dma_start(dtb[:], dtb_d[:])
            Ah = wp.tile([128, NE * DS], f32, tag="Ah")
            nc.sync.dma_start(Ah[:], Ah_d[:])
            Dh = wp.tile([128, NE], f32, tag="Dh")
            nc.sync.dma_start(Dh[:], Dh_d[:])
            Wxp = wp.tile([128, NE * 96], f16, tag="Wxp")
            nc.gpsimd.dma_start(Wxp[:], Wxp_d[:])
            Wdt = wp.tile([DTR, DI], f16, tag="Wdt")
            nc.gpsimd.dma_start(Wdt[:], Wdt_d[:])
            dt_r = wp.tile([DTR, L], f16, tag="dt_r")
            # meta = [sel, inv_sc, inv_sq, 0] broadcast to all partitions
            meta = wp.tile([128, 4], f32, tag="meta")
            nc.sync.dma_start(
                meta[:], ahi_d[512:513, 0:16]
                .bitcast(f32).partition_broadcast(128))
            sel = meta
            ident = wp.tile([128, 128], f16, tag="ident")
            nc.sync.dma_start(ident[:], eye_d[:])

            with tc.tile_pool(name="px", bufs=1) as px:
                # full-sequence x, fp16, 32 KB/part; lives phases A-C
                x_r = [px.tile([128, L], f16, tag=f"x{db}", name=f"x{db}")
                       for db in range(NK)]

                # ---------- phase A ----------
                with (tc.tile_pool(name="pa", bufs=1) as pa,
                      tc.tile_pool(name="pst", bufs=2,
                                   space="PSUM") as pst):
                    Wc = pa.tile([128, 6 * DM], f16, tag="Wc")
                    nc.gpsimd.dma_start(Wc[:], Wc_d[:])
                    segc = pa.tile([128, NK], f32, tag="segc")
                    nc.sync.dma_start(segc[:], segc_d[:])
                    segq = pa.tile([128, NK], f32, tag="segq")
                    nc.sync.dma_start(segq[:], segq_d[:])
                    csum = pa.tile([128, NK], f32, tag="csum")
                    nc.sync.dma_start(csum[:], csum_d[:])
                    # fold the code biases into effective seg biases:
                    #   segc_eff = segc - 128*inv_sc*csum
                    #   segq_eff = segq - 256*inv_sq
                    m128c = pa.tile([128, 1], f32, tag="m128c")
                    nc.vector.tensor_scalar(
                        out=m128c[:], in0=meta[:, 1:2], scalar1=-128.0,
                        scalar2=None, op0=MUL)
                    m256q = pa.tile([128, 1], f32, tag="m256q")
                    nc.vector.tensor_scalar(
                        out=m256q[:], in0=meta[:, 2:3], scalar1=-256.0,
                        scalar2=None, op0=MUL)
                    segc_eff = pa.tile([128, NK], f32, tag="segc_eff")
                    nc.vector.scalar_tensor_tensor(
                        out=segc_eff[:], in0=csum[:],
                        scalar=m128c[:, 0:1], in1=segc[:],
                        op0=MUL, op1=ADD)
                    segq_eff = pa.tile([128, NK], f32, tag="segq_eff")
                    nc.vector.tensor_scalar(
                        out=segq_eff[:], in0=segq[:],
                        scalar1=m256q[:, 0:1], scalar2=None, op0=ADD)

                    # unpack the codes in their uploaded time-major
                    # layout, then PE-transpose into feature-major tiles
                    fm = [pa.tile([128, 1024], f16, tag=f"fm{fb}",
                                  name=f"fm{fb}", bufs=1)
                          for fb in range(NA)]
                    for tb in range(8):
                        hf, r0 = tb // 4, (tb % 4) * 128
                        lot = pa.tile([128, NF], u8, tag="lot", bufs=2)
                        nc.sync.dma_start(
                            lot[:], ag_lo[hf, r0:r0 + 128, :].bitcast(u8))
                        hit = pa.tile([128, 128], u8, tag="hit", bufs=2)
                        nc.sync.dma_start(
                            hit[:], ag_hi[hf, r0:r0 + 128, :].bitcast(u8))
                        lof = pa.tile([128, NF], f16, tag="lof", bufs=2)
                        nc.scalar.copy(lof[:], lot[:])
                        hi32 = pa.tile([128, 128], i32, tag="hi32",
                                       bufs=2)
                        nc.scalar.copy(hi32[:], hit[:])
                        uq = pa.tile([128, 1024], f16, tag="uq", bufs=2)
                        for k in range(8):
                            hk = pa.tile([128, 128], i32, tag="hk",
                                         bufs=2)
                            nc.vector.tensor_scalar(
                                out=hk[:], in0=hi32[:], scalar1=k,
                                scalar2=1, op0=SHR, op1=AND)
                            hkf = pa.tile([128, 128], f16, tag="hkf",
                                          bufs=2)
                            nc.scalar.copy(hkf[:], hk[:])
                            nc.vector.scalar_tensor_tensor(
                                out=uq[:, k::8], in0=hkf[:], scalar=256.0,
                                in1=lof[:, DC + k::8], op0=MUL, op1=ADD)
                        for fb in range(NA):
                            src = (lof[:, fb * 128:(fb + 1) * 128]
                                   if fb < 6 else
                                   uq[:, (fb - 6) * 128:(fb - 5) * 128])
                            tp = pst.tile([128, 128], f16, tag="tp")
                            nc.tensor.transpose(tp[:], src, ident[:])
                            nc.scalar.copy(
                                fm[fb][:, tb * 128:(tb + 1) * 128], tp[:])
                    ctx_sb = fm[:6]
                    for db in range(NK):
                        qt = fm[6 + db]
                        cparts, qparts = [], []
                        for j in range(2):
                            jl = j * 512
                            acc = ps.tile([128, 512], f32, tag="pp")
                            for kb in range(6):
                                nc.tensor.matmul(
                                    acc[:],
                                    Wc[:, kb * DM + db * 128:
                                       kb * DM + (db + 1) * 128],
                                    ctx_sb[kb][:, jl:jl + 512],
                                    start=(kb == 0), stop=(kb == 5))
                            cp = pa.tile([128, 512], f32, tag=f"cpart{j}",
                                         name=f"cpart{j}", bufs=2)
                            nc.vector.tensor_scalar(
                                out=cp[:], in0=acc[:],
                                scalar1=meta[:, 1:2],
                                scalar2=segc_eff[:, db:db + 1],
                                op0=MUL, op1=ADD)
                            qp = pa.tile([128, 512], f32, tag=f"qpart{j}",
                                         name=f"qpart{j}", bufs=2)
                            nc.vector.tensor_scalar(
                                out=qp[:], in0=qt[:, jl:jl + 512],
                                scalar1=meta[:, 2:3],
                                scalar2=segq_eff[:, db:db + 1],
                                op0=MUL, op1=ADD)
                            cparts.append(cp)
                            qparts.append(qp)
                        for j in range(2):
                            jl = j * 512
                            # bwd (sel=0) wants time-flipped q in half0 and
                            # time-flipped c in half1: block 1-j reversed
                            crev = pa.tile([128, 512], f32, tag="crev",
                                           bufs=2)
                            nc.scalar.copy(crev[:], cparts[1 - j][:, ::-1])
                            qrev = pa.tile([128, 512], f32, tag="qrev",
                                           bufs=2)
                            nc.scalar.copy(qrev[:], qparts[1 - j][:, ::-1])
                            d0 = pa.tile([128, 512], f32, tag="d0", bufs=2)
                            nc.vector.tensor_tensor(
                                out=d0[:], in0=cparts[j][:], in1=qrev[:],
                                op=SUB)
                            s0 = pa.tile([128, 512], f32, tag="s0", bufs=2)
                            nc.vector.tensor_scalar(
                                out=s0[:], in0=d0[:], scalar1=sel[:, 0:1],
                                scalar2=None, op0=MUL)
                            nc.vector.tensor_tensor(
                                out=x_r[db][:, jl:jl + 512],
                                in0=qrev[:], in1=s0[:], op=ADD)
                            d1 = pa.tile([128, 512], f32, tag="d1", bufs=2)
                            nc.vector.tensor_tensor(
                                out=d1[:], in0=qparts[j][:], in1=crev[:],
                                op=SUB)
                            s1 = pa.tile([128, 512], f32, tag="s1", bufs=2)
                            nc.vector.tensor_scalar(
                                out=s1[:], in0=d1[:], scalar1=sel[:, 0:1],
                                scalar2=None, op0=MUL)
                            nc.vector.tensor_tensor(
                                out=x_r[db][:, Lc + jl:Lc + jl + 512],
                                in0=crev[:], in1=s1[:], op=ADD)

                # ---------- phases B/C/D ----------
                with (tc.tile_pool(name="pb", bufs=1) as pb,
                      tc.tile_pool(name="psxp", bufs=1, space="PSUM") as psxp):
                    xp_acc = [psxp.tile([96, 512], f32, tag=f"xp{tb}",
                                        name=f"xp{tb}") for tb in range(NT)]
                    for e in range(NE):
                        wt = pb.tile([128, NK * 128], f16, tag="winstream",
                                     bufs=2)
                        nc.gpsimd.dma_start(wt[:], Win_d[e, :, :])
                        upre = pb.tile([128, L + 3], f32, tag="upre", bufs=2)
                        nc.gpsimd.memset(upre[:, 0:3], 0.0)
                        for tb in range(NT):
                            acc = ps.tile([128, 512], f32, tag="pp")
                            for kb in range(NK):
                                nc.tensor.matmul(
                                    acc[:], wt[:, kb * 128:(kb + 1) * 128],
                                    x_r[kb][:, tb * 512:(tb + 1) * 512],
                                    start=(kb == 0), stop=(kb == NK - 1))
                            nc.scalar.copy(
                                upre[:, 3 + tb * 512: 3 + (tb + 1) * 512],
                                acc[:])
                        # causal depthwise conv: taps read aligned slices
                        cacc = pb.tile([128, L], f32, tag="cacc0", bufs=2)
                        nc.vector.tensor_scalar(
                            out=cacc[:], in0=upre[:, 0:L],
                            scalar1=convw[:, e * DCONV: e * DCONV + 1],
                            scalar2=None, op0=MUL)
                        for k in (1, 2, 3):
                            nxt = pb.tile([128, L], f32, tag=f"cacc{k % 2}",
                                          name=f"cacc_{k}", bufs=2)
                            nc.vector.scalar_tensor_tensor(
                                out=nxt[:], in0=upre[:, k:k + L],
                                scalar=convw[:, e * DCONV + k:
                                             e * DCONV + k + 1],
                                in1=cacc[:], op0=MUL, op1=ADD)
                            cacc = nxt
                        usilu = pb.tile([128, L], f16, tag="usilu", bufs=2)
                        nc.scalar.activation(usilu[:], cacc[:], AF.Silu,
                                             bias=convb[:, e:e + 1])
                        nc.gpsimd.dma_start(
                            u_sp[e * 128:(e + 1) * 128, :], usilu[:])
                        for tb in range(NT):
                            nc.tensor.matmul(
                                xp_acc[tb][:],
                                Wxp[:, e * 96:(e + 1) * 96],
                                usilu[:, tb * 512:(tb + 1) * 512],
                                start=(e == 0), stop=(e == NE - 1))

                    # phase C: z half -> silu -> spill
                    for e in range(NE):
                        wt = pb.tile([128, NK * 128], f16, tag="winstream",
                                     name="wtz", bufs=2)
                        nc.gpsimd.dma_start(wt[:], Win_d[NE + e, :, :])
                        for tb in range(NT):
                            acc = ps.tile([128, 512], f32, tag="pp")
                            for kb in range(NK):
                                nc.tensor.matmul(
                                    acc[:], wt[:, kb * 128:(kb + 1) * 128],
                                    x_r[kb][:, tb * 512:(tb + 1) * 512],
                                    start=(kb == 0), stop=(kb == NK - 1))
                            zt = pb.tile([128, 512], bf16, tag="zt", bufs=2)
                            nc.scalar.activation(zt[:], acc[:], AF.Silu)
                            nc.sync.dma_start(
                                zs_sp[e * 128:(e + 1) * 128,
                                      tb * 512:(tb + 1) * 512], zt[:])

                    # phase D: x_proj epilogue
                    for tb in range(NT):
                        nc.scalar.copy(dt_r[:, tb * 512:(tb + 1) * 512],
                                       xp_acc[tb][0:DTR, :])
                        bct = pb.tile([2 * DS, 512], bf16, tag="bct", bufs=2)
                        nc.scalar.copy(bct[:], xp_acc[tb][DTR:96, :])
                        nc.sync.dma_start(
                            bc_sp[:, tb * 512:(tb + 1) * 512], bct[:])

            # ---------- phase E: dt_proj -> delta, dg ----------
            with tc.tile_pool(name="pe", bufs=1) as pe:
                for e in range(NE):
                    delta = pe.tile([128, L], f32, tag="delta", bufs=2)
                    for tb in range(NT):
                        acc = ps.tile([128, 512], f32, tag="pp")
                        nc.tensor.matmul(
                            acc[:], Wdt[:, e * 128:(e + 1) * 128],
                            dt_r[:, tb * 512:(tb + 1) * 512],
                            start=True, stop=True)
                        # softplus(x + b) = ln(1 + exp(x + b)); inputs here
                        # are small (|x|<6) so exp cannot overflow
                        ex = pe.tile([128, 512], f32, tag="spexp", bufs=2)
                        nc.scalar.activation(
                            ex[:], acc[:], AF.Exp, bias=dtb[:, e:e + 1])
                        nc.scalar.activation(
                            delta[:, tb * 512:(tb + 1) * 512], ex[:],
                            AF.Ln, bias=1.0)
                    nc.gpsimd.dma_start(
                        dl_sp[e * 128:(e + 1) * 128, :], delta[:])
                    ub = pe.tile([128, L], f16, tag="ub_e", bufs=2)
                    nc.sync.dma_start(ub[:], u_sp[e * 128:(e + 1) * 128, :])
                    dg = pe.tile([128, L], f16, tag="dg_e", bufs=2)
                    nc.vector.tensor_tensor(out=dg[:], in0=delta[:],
                                            in1=ub[:], op=MUL)
                    nc.sync.dma_start(
                        dg_sp[e * 128:(e + 1) * 128, :], dg[:])

            # ---------- phase F: selective scan ----------
            with tc.tile_pool(name="pf", bufs=1) as pf:
                for p in range(2):
                    Bb, Cb = [], []
                    for si in range(8):
                        s = p * 8 + si
                        bb = pf.tile([128, L], bf16, tag=f"Bb{si}",
                                     name=f"Bb{si}")
                        nc.sync.dma_start(
                            bb[:], bc_sp[s:s + 1, :].partition_broadcast(128))
                        cb = pf.tile([128, L], bf16, tag=f"Cb{si}",
                                     name=f"Cb{si}")
                        nc.sync.dma_start(
                            cb[:],
                            bc_sp[DS + s:DS + s + 1, :].partition_broadcast(128))
                        Bb.append(bb)
                        Cb.append(cb)
                    for e in range(NE):
                        dl = pf.tile([128, L], f16, tag="dl_f", bufs=2)
                        nc.sync.dma_start(
                            dl[:], dl_sp[e * 128:(e + 1) * 128, :])
                        dgt = pf.tile([128, L], f16, tag="dg_f", bufs=2)
                        nc.sync.dma_start(
                            dgt[:], dg_sp[e * 128:(e + 1) * 128, :])
                        if p == 0:
                            ub = pf.tile([128, L], f16, tag="ub_f", bufs=2)
                            nc.sync.dma_start(
                                ub[:], u_sp[e * 128:(e + 1) * 128, :])
                            yacc = pf.tile([128, L], f32, tag="yacc0",
                                           name="yacc_i", bufs=1)
                            nc.vector.tensor_scalar(
                                out=yacc[:], in0=ub[:],
                                scalar1=Dh[:, e:e + 1], scalar2=None, op0=MUL)
                        else:
                            yacc = pf.tile([128, L], f32, tag="yacc0",
                                           name="yacc_l", bufs=1)
                            nc.sync.dma_start(
                                yacc[:], yacc_sp[e * 128:(e + 1) * 128, :])
                        for si in range(8):
                            s = p * 8 + si
                            dA = pf.tile([128, L], f32, tag="dA", bufs=2)
                            nc.scalar.activation(
                                dA[:], dl[:], AF.Exp,
                                scale=Ah[:, e * DS + s: e * DS + s + 1])
                            dgB = pf.tile([128, L], bf16, tag="dgB", bufs=2)
                            nc.vector.tensor_tensor(
                                out=dgB[:], in0=dgt[:], in1=Bb[si][:], op=MUL)
                            h = pf.tile([128, L], bf16, tag="h", bufs=2)
                            nc.vector.tensor_tensor_scan(
                                h[:], dA[:], dgB[:], 0.0, op0=MUL, op1=ADD)
                            ch = pf.tile([128, L], bf16, tag="ch", bufs=2)
                            nc.vector.tensor_tensor(
                                out=ch[:], in0=h[:], in1=Cb[si][:], op=MUL)
                            ynew = pf.tile([128, L], f32,
                                           tag=f"yacc{(si + 1) % 2}",
                                           name=f"yacc_{si}", bufs=1)
                            nc.gpsimd.tensor_tensor(
                                out=ynew[:], in0=yacc[:], in1=ch[:], op=ADD)
                            yacc = ynew
                        if p == 0:
                            nc.sync.dma_start(
                                yacc_sp[e * 128:(e + 1) * 128, :], yacc[:])
                        else:
                            zst = pf.tile([128, L], bf16, tag="zs_f", bufs=2)
                            nc.sync.dma_start(
                                zst[:], zs_sp[e * 128:(e + 1) * 128, :])
                            yg = pf.tile([128, L], f16, tag="yg", bufs=2)
                            nc.vector.tensor_tensor(
                                out=yg[:], in0=yacc[:], in1=zst[:], op=MUL)
                            nc.sync.dma_start(
                                yg_sp[e * 128:(e + 1) * 128, :], yg[:])

            # ---------- phase G: out_proj on the sel-chosen half ----------
            # out_proj runs operand-swapped so og is TIME-major:
            # og[t, d] = sum_di ysel[di, t] * Wout[d, di]
            with tc.tile_pool(name="pg", bufs=1) as pg:
                Wout = pg.tile([128, NE * DM], f16, tag="Wout")
                nc.gpsimd.dma_start(Wout[:], Wout_d[:])
                ysel = [pg.tile([128, Lq], f16, tag=f"ys{kb}",
                                name=f"ys{kb}", bufs=1) for kb in range(NE)]
                for j in range(2):
                    jl = j * 512
                    for kb in range(NE):
                        # fwd (sel=1): natural cols Lc+jl..; bwd (sel=0):
                        # cols (1-j)*512.. time-reversed
                        ylo = pg.tile([128, 512], f16, tag="ylo", bufs=2)
                        nc.sync.dma_start(
                            ylo[:], yg_sp[kb * 128:(kb + 1) * 128,
                                          (1 - j) * 512:(2 - j) * 512])
                        yhi = pg.tile([128, 512], f16, tag="yhi", bufs=2)
                        nc.sync.dma_start(
                            yhi[:], yg_sp[kb * 128:(kb + 1) * 128,
                                          Lc + jl:Lc + jl + 512])
                        yrev = pg.tile([128, 512], f16, tag="yrev", bufs=2)
                        nc.scalar.copy(yrev[:], ylo[:, ::-1])
                        dft = pg.tile([128, 512], f32, tag="dft", bufs=2)
                        nc.vector.tensor_tensor(
                            out=dft[:], in0=yhi[:], in1=yrev[:], op=SUB)
                        sdf = pg.tile([128, 512], f32, tag="sdf", bufs=2)
                        nc.vector.tensor_scalar(
                            out=sdf[:], in0=dft[:], scalar1=sel[:, 0:1],
                            scalar2=None, op0=MUL)
                        nc.vector.tensor_tensor(
                            out=ysel[kb][:, jl:jl + 512], in0=yrev[:],
                            in1=sdf[:], op=ADD)
                for tb in range(8):
                    for dh in range(2):
                        acc = ps.tile([128, 512], f32, tag="pp")
                        for kb in range(NE):
                            nc.tensor.matmul(
                                acc[:],
                                ysel[kb][:, tb * 128:(tb + 1) * 128],
                                Wout[:, kb * DM + dh * 512:
                                     kb * DM + dh * 512 + 512],
                                start=(kb == 0), stop=(kb == NE - 1))
                        ot = pg.tile([128, 512], f16, tag="ot", bufs=2)
                        nc.scalar.copy(ot[:], acc[:])
                        nc.sync.dma_start(
                            og_sp[tb * 128:(tb + 1) * 128,
                                  dh * 512:dh * 512 + 512], ot[:])

                # pair AllReduce: fwd+bwd sum lands on both cores
                nc.gpsimd.collective_compute(
                    "AllReduce", ADD, replica_groups=PAIRS,
                    ins=[og_sp[:].opt()], outs=[og_sum[:].opt()])

                # each core outputs its dm-half (fwd cols 0:512, bwd
                # 512:1024), int8-quantized with a per-time-row f32
                # scale (scale includes the 0.5 fwd/bwd average factor)
                for tb in range(8):
                    r0, r1 = tb * 128, (tb + 1) * 128
                    stp = pg.tile([128, 512], f16, tag="stp", bufs=2)
                    nc.sync.dma_start(stp[:], og_sum[r0:r1, 0:512])
                    sbt = pg.tile([128, 512], f16, tag="sbt", bufs=2)
                    nc.sync.dma_start(sbt[:], og_sum[r0:r1, 512:1024])
                    dfo = pg.tile([128, 512], f32, tag="dfo", bufs=2)
                    nc.vector.tensor_tensor(
                        out=dfo[:], in0=stp[:], in1=sbt[:], op=SUB)
                    sfo = pg.tile([128, 512], f32, tag="sfo", bufs=2)
                    nc.vector.tensor_scalar(
                        out=sfo[:], in0=dfo[:], scalar1=sel[:, 0:1],
                        scalar2=None, op0=MUL)
                    oh = pg.tile([128, 512], f32, tag="oh", bufs=2)
                    nc.vector.tensor_tensor(
                        out=oh[:], in0=sbt[:], in1=sfo[:], op=ADD)
                    rmax = pg.tile([128, 1], f32, tag="rmax", bufs=2)
                    nc.vector.tensor_reduce(
                        out=rmax[:], in_=oh[:], axis=AX.X, op=MAXO,
                        apply_absolute_value=True)
                    srow = pg.tile([128, 1], f32, tag="srow", bufs=2)
                    nc.vector.tensor_scalar(
                        out=srow[:], in0=rmax[:], scalar1=1e-30,
                        scalar2=1.0 / 127, op0=MAXO, op1=MUL)
                    rinv = pg.tile([128, 1], f32, tag="rinv", bufs=2)
                    nc.vector.reciprocal(rinv[:], srow[:])
                    sdl = pg.tile([128, 1], f32, tag="sdl", bufs=2)
                    nc.vector.tensor_scalar(
                        out=sdl[:], in0=srow[:], scalar1=0.5,
                        scalar2=None, op0=MUL)
                    q8 = pg.tile([128, 512], i8, tag="q8", bufs=2)
                    nc.scalar.activation(q8[:], oh[:], AF.Copy,
                                         scale=rinv[:, 0:1])
                    nc.sync.dma_start(out_d[r0:r1, 0:512],
                                      q8[:].bitcast(u8))
                    nc.sync.dma_start(out_d[r0:r1, 512:516],
                                      sdl[:].bitcast(u8))

    nc.compile()
    return nc


def _install_cc_cache():
    """Content-keyed disk cache around the neuronx compiler hook.

    The bass_exec compile path (walrus) takes ~60 s for this program and
    has no persistent cache of its own; the emitted BIR (and hence the
    HLO carrying it) is byte-deterministic, so a sha256-of-HLO keyed
    cache makes every process after the first skip the compile.
    """
    import os
    try:
        import libneuronxla
    except ImportError:
        return
    if getattr(libneuronxla, "_bass_cc_disk_cache", False):
        return
    inner = libneuronxla.neuronx_cc
    cache_dir = os.environ.get(
        "NEURON_COMPILE_CACHE_URL",
        os.path.join(os.path.expanduser("~"), ".neuron-compile-cache"))
    try:
        os.makedirs(cache_dir, exist_ok=True)
    except OSError:
        libneuronxla._bass_cc_disk_cache = True
        return

    def cached(code, code_format, platform_version, file_prefix,
               *a, **kw):
        c = code if isinstance(code, (bytes, bytearray)) else \
            str(code).encode()
        key = hashlib.sha256(
            c + b"|" + str(platform_version).encode()).hexdigest()
        path = os.path.join(cache_dir, f"bassneff-{key}.hlo")
        try:
            with open(path, "rb") as f:
                return 0, f.read()
        except OSError:
            pass
        r = inner(code, code_format, platform_version, file_prefix,
                  *a, **kw)
        try:
            err, blob = r
            if err == 0 and isinstance(blob, (bytes, bytearray)) and blob:
                tmp = f"{path}.tmp.{os.getpid()}"
                with open(tmp, "wb") as f:
                    f.write(blob)
                os.replace(tmp, path)
        except Exception:
            pass
        return r

    libneuronxla.neuronx_cc = cached
    libneuronxla._bass_cc_disk_cache = True


def _make_runner(nc):
    """Jit the SPMD dispatch once; repeat calls hit the C++ fast path.

    Mirrors what bass_utils.run_bass_kernel_spmd does under axon
    (bass2jax.run_bass_via_pjrt), minus the per-call re-jit and the
    donated zero output buffers (the kernel writes every output
    element, so uninitialized outputs are fine).
    """
    import jax
    import numpy as np
    from jax.sharding import Mesh, PartitionSpec
    from jax.experimental.shard_map import shard_map
    from concourse import mybir
    from concourse.bass2jax import (_bass_exec_p, install_neuronx_cc_hook,
                                    partition_id_tensor)

    install_neuronx_cc_hook()
    _install_cc_cache()
    partition_name = (nc.partition_id_tensor.name
                      if nc.partition_id_tensor else None)
    in_names, out_names, out_avals = [], [], []
    for alloc in nc.m.functions[0].allocations:
        if not isinstance(alloc, mybir.MemoryLocationSet):
            continue
        name = alloc.memorylocations[0].name
        if alloc.kind == "ExternalInput":
            if name != partition_name:
                in_names.append(name)
        elif alloc.kind == "ExternalOutput":
            out_names.append(name)
            out_avals.append(jax.core.ShapedArray(
                tuple(alloc.tensor_shape), mybir.dt.np(alloc.dtype)))
    in_names_all = list(in_names)
    if partition_name is not None:
        in_names_all.append(partition_name)

    def _body(*args):
        operands = list(args)
        if partition_name is not None:
            operands.append(partition_id_tensor())
        return tuple(_bass_exec_p.bind(
            *operands, out_avals=tuple(out_avals),
            in_names=tuple(in_names_all), out_names=tuple(out_names),
            lowering_input_output_aliases=(),
            sim_require_finite=True, sim_require_nnan=True, nc=nc))

    devices = jax.devices()[:NCORE]
    mesh = Mesh(np.asarray(devices), ("core",))
    n_in = len(in_names)
    sharded = jax.jit(shard_map(
        _body, mesh=mesh, in_specs=(PartitionSpec("core"),) * n_in,
        out_specs=(PartitionSpec("core"),) * len(out_names),
        check_rep=False))

    def run(per_name_globals):
        args = [per_name_globals[name] for name in in_names]
        outs = sharded(*args)
        return {name: outs[i] for i, name in enumerate(out_names)}

    return run


_NF = NA * 128
_BUF = None
_MESH = None


def _mesh():
    global _MESH
    if _MESH is None:
        import jax
        from jax.sharding import Mesh, PartitionSpec, NamedSharding
        devs = jax.devices()[:NCORE]
        m = Mesh(np.asarray(devs), ("core",))
        _MESH = (m, devs, NamedSharding(m, PartitionSpec("core")))
    return _MESH


def _bufs():
    global _BUF
    if _BUF is None:
        _BUF = dict(
            glo=np.empty((NCORE, 512, _NF), np.uint8),
            ghi=np.empty((NCORE, 513, 128), np.uint8),
            u16=np.empty((512, DQ), np.uint16),
            f32=np.empty((512, _NF), np.float32),
            y=[np.zeros((B, Lq, DM), np.float32) for _ in range(2)],
            yi=0,
        )
    return _BUF


def kernel(**inputs) -> np.ndarray:
    global _RUN, _WFP
    bufs = _bufs()
    fp = _fingerprint(inputs)
    if _RUN is None or fp != _WFP:
        _RUN = _make_runner(_build(_prep_weights(inputs)))
        _WFP = fp

    qf = np.asarray(inputs["query"], np.float32)
    cf_ = np.asarray(inputs["context"], np.float32)
    # ctx: 8-bit code u = round(x*s_c)+128; q: 9-bit u = round(x*s_q)+256
    # all host passes run on contiguous views (single CPU core)
    s_q = 255.0 / max(np.abs(qf).max(), 1e-30)
    s_c = 127.0 / max(np.abs(cf_).max(), 1e-30)
    glo, ghi = bufs["glo"], bufs["ghi"]
    u16, ftmp = bufs["u16"], bufs["f32"]
    meta = np.zeros((NCORE, 4), np.float32)
    meta[:4, 0] = 1.0
    meta[:, 1] = 1.0 / s_c
    meta[:, 2] = 1.0 / s_q
    mbytes = meta.view(np.uint8).reshape(NCORE, 16)
    # device_put per core as soon as it is packed: puts are async, so
    # the tunnel upload streams while the host packs the next core
    import jax
    devs, shards_lo, shards_hi = _mesh()[1], [], []
    for core in range(NCORE):
        b, hf = core % 4, core // 4
        hs = slice(hf * 512, (hf + 1) * 512)
        np.multiply(cf_[b, hs], s_c, out=ftmp[:, :DC])
        np.add(ftmp[:, :DC], 128.5, out=glo[core][:, :DC],
               casting="unsafe")
        np.multiply(qf[b, hs], s_q, out=ftmp[:, DC:])
        np.add(ftmp[:, DC:], 256.5, out=u16, casting="unsafe")
        np.bitwise_and(u16, 255, out=glo[core][:, DC:], casting="unsafe")
        hb = (u16 >> 8).astype(np.uint8).reshape(512, 128, 8)
        ghi[core, :512] = np.packbits(hb, axis=2, bitorder="little")[:, :, 0]
        ghi[core, 512, :16] = mbytes[core]
        shards_lo.append(jax.device_put(glo[core], devs[core]))
        shards_hi.append(jax.device_put(ghi[core], devs[core]))

    mesh_sh = _mesh()[2]
    ga = jax.make_array_from_single_device_arrays(
        (NCORE * 512, _NF), mesh_sh, shards_lo)
    gh = jax.make_array_from_single_device_arrays(
        (NCORE * 513, 128), mesh_sh, shards_hi)
    res = _RUN({"alo": ga, "ahi": gh})
    o = np.asarray(res["out"]).reshape(NCORE, Lq, DM // 2 + 4)
    q8 = o[:, :, :DM // 2].view(np.int8)
    sc = np.ascontiguousarray(o[:, :, DM // 2:]).view(np.float32)
    bufs["yi"] ^= 1
    y = bufs["y"][bufs["yi"]]
    for b in range(B):
        np.multiply(q8[b], sc[b], out=y[b, :, :DM // 2])
        np.multiply(q8[4 + b], sc[4 + b], out=y[b, :, DM // 2:])
    return y



# revision 10
# speedup vs baseline: 1.0872x; 1.0872x over previous
"""CrossMamba Trainium2 kernel.

Sharding: 8 cores = 4 batches x 2 scan directions. Core b and core 4+b
form a pair that works on batch b; both run the same SPMD program and
differ only in a 4-byte selector in the meta row (sel=1 fwd, 0 bwd).

Wall-clock is dominated by the axon tunnel (~30-100 MB/s shared, ~80 ms
fixed latency per direction; device exec is only ~3.6 ms), so the I/O
contract is tuned for wire bytes and host (single-CPU) cost:
  - all weights are baked into the NEFF as inline Const tensors
    (transferred once at executable load, never per call)
  - activations upload quantized: ctx as an 8-bit code (its noise is
    averaged down by the 768-wide c_in GEMM), q as a 9-bit code
    (low byte + bit-packed high bit). 0.94 MB per core, 7.5 MB total.
    Codes are packed host-side in natural [time, feature] layout with
    contiguous numpy ops only; the device unpacks (int shift/and ops)
    and PE-transposes to feature-major. The +2^(b-1) code bias is
    folded into effective seg biases (for ctx via colsum(c_in_w)), so
    dequantization costs no extra per-element work
  - each core uploads only HALF of its batch's sequence; an on-device
    pair AllGather (over f32-viewed byte buffers) reassembles it
  - per-core shards are device_put as soon as they are packed (puts
    are async), overlapping host packing with the upload stream; the
    output is fetched without an intermediate block_until_ready, which
    hides the execute round trip
  - the backward direction is derived on device: exact 0/1 sel-blends
    choose operand placement, and negative-stride (reversed-AP) copies
    time-flip the data, so fwd and bwd cores run one program
  - out_proj runs operand-swapped so the output is TIME-major, the
    fwd+bwd results are summed with a pair AllReduce, and each core
    downloads its dm-half int8-quantized with a per-time-row f32 scale
    (scale carries the 0.5 fwd/bwd average): 0.53 MB per core, 4.3 MB
    total. Host dequant is two contiguous broadcasts per batch
  - the jitted executable is cached at module level keyed on a weight
    fingerprint; repeat calls skip re-trace/re-compile

Per-core program:
  A0) bounce upload to DRAM, pair AllGather -> full time range
  A) unpack codes, PE-transpose, x = blend(c_in(ctx)+segc', q+segq')
     with sel-driven placement/flip
  B) in_proj (u half) -> causal depthwise conv -> silu -> x_proj acc
  C) in_proj (z half) -> silu -> spill
  D) x_proj epilogue (dt / B / C rows)
  E) dt_proj -> softplus -> delta, dg = delta*u
  F) selective scan: per (channel-block, state): dA = exp(A_s*delta),
     dgB, hardware tensor_tensor_scan, C-readout, state accumulation;
     two passes of 8 states
  G) gate with silu(z), sel-chosen/flipped time half -> time-major
     out_proj, pair AllReduce, int8 quantize, output the dm half

GEMMs run in fp16 (f32 PSUM accumulate), scan math in f32/bf16.
End-to-end relative error vs the fp32 reference: ~9e-3 (gate: 2e-2).
"""
import hashlib
import numpy as np

B, Lq, Lc = 4, 1024, 1024
DQ, DC, DM = 1024, 768, 1024
DS, DCONV = 16, 4
DI, DTR = 2048, 64
L = Lc + Lq              # 2048
NCORE = 8
NE = DI // 128           # 16 u (or z) channel blocks
NK = DM // 128           # 8 k blocks for in_proj
NT = L // 512            # 4 time blocks of 512
NA = (DC + DM) // 128    # 14 row blocks in the packed activation half

_RUN = None              # cached (runner, weight fingerprint)
_WFP = None

_WKEYS = ("c_in_w", "seg_context", "seg_query", "in_proj_w", "conv_w",
          "conv_b", "x_proj_w", "dt_proj_w", "dt_proj_b", "A_log", "D",
          "mamba_out_w")


def _fingerprint(inputs):
    h = hashlib.blake2b(digest_size=16)
    for k in _WKEYS:
        a = np.ascontiguousarray(np.asarray(inputs[k]))
        h.update(k.encode())
        h.update(str(a.shape).encode())
        b = a.view(np.uint8).reshape(-1)
        step = max(1, b.size // 65536)
        h.update(bytes(b[::step][:65536]))
    return h.digest()


def _prep_weights(inputs):
    f32, f16 = np.float32, np.float16
    c_in_w = np.asarray(inputs["c_in_w"], f32)
    segc = np.asarray(inputs["seg_context"], f32).reshape(DM)
    segq = np.asarray(inputs["seg_query"], f32).reshape(DM)
    in_proj_w = np.asarray(inputs["in_proj_w"], f32)
    conv_w = np.asarray(inputs["conv_w"], f32)
    conv_b = np.asarray(inputs["conv_b"], f32)
    x_proj_w = np.asarray(inputs["x_proj_w"], f32)
    dt_proj_w = np.asarray(inputs["dt_proj_w"], f32)
    dt_proj_b = np.asarray(inputs["dt_proj_b"], f32)
    A = (-np.exp(np.asarray(inputs["A_log"], f32))).astype(f32)
    D = np.asarray(inputs["D"], f32)
    out_w = np.asarray(inputs["mamba_out_w"], f32)

    def blk(a, p=128):
        # [n*p, m] -> [p, n*m] with n-major free layout
        n = a.shape[0] // p
        return np.ascontiguousarray(
            a.reshape(n, p, -1).transpose(1, 0, 2).reshape(p, -1))

    return dict(
        Wc=blk(c_in_w.T).astype(f16),                     # [128, 6*1024]
        segc=np.ascontiguousarray(segc.reshape(NK, 128).T),   # [128, 8]
        segq=np.ascontiguousarray(segq.reshape(NK, 128).T),
        # row sums of c_in_w (over the DC axis): used to fold the
        # +512 bias of the 10-bit activation code out of the c_in GEMM
        csum=np.ascontiguousarray(
            c_in_w.sum(axis=1).reshape(NK, 128).T),       # [128, 8]
        Win=np.ascontiguousarray(
            in_proj_w.reshape(32, 128, NK, 128).transpose(0, 3, 2, 1)
            .reshape(32, 128, NK * 128)).astype(f16),     # [32,128,1024]
        Wxp=blk(x_proj_w.T).astype(f16),                  # [128, 16*96]
        Wdt=np.ascontiguousarray(dt_proj_w.T).astype(f16),  # [64, 2048]
        Wout=np.ascontiguousarray(
            out_w.reshape(8, 128, NE, 128).transpose(3, 2, 0, 1)
            .reshape(128, NE * DM)).astype(f16),          # [128, 16*1024]
        convw=blk(conv_w),                                # [128, 16*4]
        convb=conv_b.reshape(NE, 128).T.copy(),
        dtb=dt_proj_b.reshape(NE, 128).T.copy(),
        Ah=blk(A),                                        # [128, 16*16]
        Dh=D.reshape(NE, 128).T.copy(),
    )


def _build(w):
    import concourse.bacc as bacc
    import concourse.tile as tile
    from concourse import mybir

    f32 = mybir.dt.float32
    f16 = mybir.dt.float16
    bf16 = mybir.dt.bfloat16
    u8 = mybir.dt.uint8
    i8 = mybir.dt.int8
    i32 = mybir.dt.int32
    MUL = mybir.AluOpType.mult
    ADD = mybir.AluOpType.add
    SUB = mybir.AluOpType.subtract
    BYP = mybir.AluOpType.bypass
    MAXO = mybir.AluOpType.max
    SHR = mybir.AluOpType.logical_shift_right
    AND = mybir.AluOpType.bitwise_and
    AF = mybir.ActivationFunctionType
    AX = mybir.AxisListType
    PAIRS = [[0, 4], [1, 5], [2, 6], [3, 7]]

    nc = bacc.Bacc("TRN2", target_bir_lowering=False, debug=False,
                   num_devices=NCORE)

    # ---- per-core external inputs ----
    # Quantized activations for this core's half of the batch's [ctx, q]
    # feature-concat, NATURAL [time, feature] layout (host packs with
    # contiguous ops only; the device transposes). Core b carries times
    # 0:512, core 4+b times 512:1024.
    # ctx uses an 8-bit code u = round(x*s_c)+128 (its quantization
    # noise is averaged down by the 768-wide c_in GEMM); q uses a 9-bit
    # code u = round(x*s_q)+256 (it enters x directly).
    # One combined tensor per core (one put, one AllGather):
    #   rows 0:512: ctx code bytes (cols 0:768) | q low bytes (cols
    #     768:1792) | q high bits, 8 consecutive features per byte,
    #     little bit order (cols 1792:1920)
    #   row 512: 16 meta bytes = f32 [sel, 1/s_ctx, 1/s_q, 0]
    NF = NA * 128            # 1792 features (ctx 768 | q 1024)
    NW = NF + 128            # +128 hi-bit bytes
    cmb_d = nc.dram_tensor("acmb", [513, NW], u8, kind="ExternalInput")

    # ---- weights baked into the NEFF (loaded once, not per call) ----
    Wc_d = nc.inline_tensor(w["Wc"], name="Wc_i")
    segc_d = nc.inline_tensor(w["segc"], name="segc_i")
    segq_d = nc.inline_tensor(w["segq"], name="segq_i")
    csum_d = nc.inline_tensor(w["csum"], name="csum_i")
    Win_d = nc.inline_tensor(w["Win"], name="Win_i")
    Wxp_d = nc.inline_tensor(w["Wxp"], name="Wxp_i")
    Wdt_d = nc.inline_tensor(w["Wdt"], name="Wdt_i")
    Wout_d = nc.inline_tensor(w["Wout"], name="Wout_i")
    convw_d = nc.inline_tensor(w["convw"], name="convw_i")
    convb_d = nc.inline_tensor(w["convb"], name="convb_i")
    dtb_d = nc.inline_tensor(w["dtb"], name="dtb_i")
    Ah_d = nc.inline_tensor(w["Ah"], name="Ah_i")
    Dh_d = nc.inline_tensor(w["Dh"], name="Dh_i")
    eye_d = nc.inline_tensor(np.eye(128, dtype=np.float16), name="eye_i")

    # ---- DRAM scratch ----
    # bounce/gather buffers are declared f32 (collective-safe dtype) and
    # byte-addressed via bitcast; widths are bytes/4
    cmb_bnc = nc.dram_tensor("cmb_bnc", [512, NW // 4], f32)
    ag_cmb = nc.dram_tensor("ag_cmb", [2, 512, NW // 4], f32)
    u_sp = nc.dram_tensor("u_sp", [DI, L], f16)
    zs_sp = nc.dram_tensor("zs_sp", [DI, L], bf16)
    dl_sp = nc.dram_tensor("dl_sp", [DI, L], f16)
    dg_sp = nc.dram_tensor("dg_sp", [DI, L], f16)
    bc_sp = nc.dram_tensor("bc_sp", [2 * DS, L], bf16)
    yacc_sp = nc.dram_tensor("yacc_sp", [DI, L], f32)
    yg_sp = nc.dram_tensor("yg_sp", [DI, L], f16)
    og_sp = nc.dram_tensor("og_sp", [Lq, DM], f16)
    og_sum = nc.dram_tensor("og_sum", [Lq, DM], f16)

    # time-major int8 output + per-time-row f32 scale in cols 512:516
    out_d = nc.dram_tensor("out", [Lq, DM // 2 + 4], u8,
                           kind="ExternalOutput")

    with tile.TileContext(nc) as tc:
        with (
            tc.tile_pool(name="wp", bufs=1) as wp,
            tc.tile_pool(name="ps", bufs=3, space="PSUM") as ps,
        ):
            # ---------- phase A0: bounce + pair AllGather ----------
            with tc.tile_pool(name="p0", bufs=2) as p0:
                for rb in range(4):
                    r0, r1 = rb * 128, (rb + 1) * 128
                    bt = p0.tile([128, NW], u8, tag="bnc")
                    nc.sync.dma_start(bt[:], cmb_d[r0:r1, :])
                    nc.sync.dma_start(cmb_bnc[r0:r1, :].bitcast(u8), bt[:])
            nc.gpsimd.collective_compute(
                "AllGather", BYP, replica_groups=PAIRS,
                ins=[cmb_bnc[:].opt()], outs=[ag_cmb[:].opt()])

            # ---------- small persistent weights ----------
            convw = wp.tile([128, NE * DCONV], f32, tag="convw")
            nc.sync.dma_start(convw[:], convw_d[:])
            convb = wp.tile([128, NE], f32, tag="convb")
            nc.sync.dma_start(convb[:], convb_d[:])
            dtb = wp.tile([128, NE], f32, tag="dtb")
            nc.sync.dma_start(dtb[:], dtb_d[:])
            Ah = wp.tile([128, NE * DS], f32, tag="Ah")
            nc.sync.dma_start(Ah[:], Ah_d[:])
            Dh = wp.tile([128, NE], f32, tag="Dh")
            nc.sync.dma_start(Dh[:], Dh_d[:])
            Wxp = wp.tile([128, NE * 96], f16, tag="Wxp")
            nc.gpsimd.dma_start(Wxp[:], Wxp_d[:])
            Wdt = wp.tile([DTR, DI], f16, tag="Wdt")
            nc.gpsimd.dma_start(Wdt[:], Wdt_d[:])
            dt_r = wp.tile([DTR, L], f16, tag="dt_r")
            # meta = [sel, inv_sc, inv_sq, 0] broadcast to all partitions
            meta = wp.tile([128, 4], f32, tag="meta")
            nc.sync.dma_start(
                meta[:], cmb_d[512:513, 0:16]
                .bitcast(f32).partition_broadcast(128))
            sel = meta
            ident = wp.tile([128, 128], f16, tag="ident")
            nc.sync.dma_start(ident[:], eye_d[:])

            with tc.tile_pool(name="px", bufs=1) as px:
                # full-sequence x, fp16, 32 KB/part; lives phases A-C
                x_r = [px.tile([128, L], f16, tag=f"x{db}", name=f"x{db}")
                       for db in range(NK)]

                # ---------- phase A ----------
                with (tc.tile_pool(name="pa", bufs=1) as pa,
                      tc.tile_pool(name="pst", bufs=2,
                                   space="PSUM") as pst):
                    Wc = pa.tile([128, 6 * DM], f16, tag="Wc")
                    nc.gpsimd.dma_start(Wc[:], Wc_d[:])
                    segc = pa.tile([128, NK], f32, tag="segc")
                    nc.sync.dma_start(segc[:], segc_d[:])
                    segq = pa.tile([128, NK], f32, tag="segq")
                    nc.sync.dma_start(segq[:], segq_d[:])
                    csum = pa.tile([128, NK], f32, tag="csum")
                    nc.sync.dma_start(csum[:], csum_d[:])
                    # fold the code biases into effective seg biases:
                    #   segc_eff = segc - 128*inv_sc*csum
                    #   segq_eff = segq - 256*inv_sq
                    m128c = pa.tile([128, 1], f32, tag="m128c")
                    nc.vector.tensor_scalar(
                        out=m128c[:], in0=meta[:, 1:2], scalar1=-128.0,
                        scalar2=None, op0=MUL)
                    m256q = pa.tile([128, 1], f32, tag="m256q")
                    nc.vector.tensor_scalar(
                        out=m256q[:], in0=meta[:, 2:3], scalar1=-256.0,
                        scalar2=None, op0=MUL)
                    segc_eff = pa.tile([128, NK], f32, tag="segc_eff")
                    nc.vector.scalar_tensor_tensor(
                        out=segc_eff[:], in0=csum[:],
                        scalar=m128c[:, 0:1], in1=segc[:],
                        op0=MUL, op1=ADD)
                    segq_eff = pa.tile([128, NK], f32, tag="segq_eff")
                    nc.vector.tensor_scalar(
                        out=segq_eff[:], in0=segq[:],
                        scalar1=m256q[:, 0:1], scalar2=None, op0=ADD)

                    # unpack the codes in their uploaded time-major
                    # layout, then PE-transpose into feature-major tiles
                    fm = [pa.tile([128, 1024], f16, tag=f"fm{fb}",
                                  name=f"fm{fb}", bufs=1)
                          for fb in range(NA)]
                    for tb in range(8):
                        hf, r0 = tb // 4, (tb % 4) * 128
                        lot = pa.tile([128, NF], u8, tag="lot", bufs=2)
                        nc.sync.dma_start(
                            lot[:],
                            ag_cmb[hf, r0:r0 + 128, 0:NF // 4].bitcast(u8))
                        hit = pa.tile([128, 128], u8, tag="hit", bufs=2)
                        nc.sync.dma_start(
                            hit[:],
                            ag_cmb[hf, r0:r0 + 128,
                                   NF // 4:NW // 4].bitcast(u8))
                        lof = pa.tile([128, NF], f16, tag="lof", bufs=2)
                        nc.scalar.copy(lof[:], lot[:])
                        hi32 = pa.tile([128, 128], i32, tag="hi32",
                                       bufs=2)
                        nc.scalar.copy(hi32[:], hit[:])
                        uq = pa.tile([128, 1024], f16, tag="uq", bufs=2)
                        for k in range(8):
                            hk = pa.tile([128, 128], i32, tag="hk",
                                         bufs=2)
                            nc.vector.tensor_scalar(
                                out=hk[:], in0=hi32[:], scalar1=k,
                                scalar2=1, op0=SHR, op1=AND)
                            hkf = pa.tile([128, 128], f16, tag="hkf",
                                          bufs=2)
                            nc.scalar.copy(hkf[:], hk[:])
                            nc.vector.scalar_tensor_tensor(
                                out=uq[:, k::8], in0=hkf[:], scalar=256.0,
                                in1=lof[:, DC + k::8], op0=MUL, op1=ADD)
                        for fb in range(NA):
                            src = (lof[:, fb * 128:(fb + 1) * 128]
                                   if fb < 6 else
                                   uq[:, (fb - 6) * 128:(fb - 5) * 128])
                            tp = pst.tile([128, 128], f16, tag="tp")
                            nc.tensor.transpose(tp[:], src, ident[:])
                            nc.scalar.copy(
                                fm[fb][:, tb * 128:(tb + 1) * 128], tp[:])
                    ctx_sb = fm[:6]
                    for db in range(NK):
                        qt = fm[6 + db]
                        cparts, qparts = [], []
                        for j in range(2):
                            jl = j * 512
                            acc = ps.tile([128, 512], f32, tag="pp")
                            for kb in range(6):
                                nc.tensor.matmul(
                                    acc[:],
                                    Wc[:, kb * DM + db * 128:
                                       kb * DM + (db + 1) * 128],
                                    ctx_sb[kb][:, jl:jl + 512],
                                    start=(kb == 0), stop=(kb == 5))
                            cp = pa.tile([128, 512], f32, tag=f"cpart{j}",
                                         name=f"cpart{j}", bufs=2)
                            nc.vector.tensor_scalar(
                                out=cp[:], in0=acc[:],
                                scalar1=meta[:, 1:2],
                                scalar2=segc_eff[:, db:db + 1],
                                op0=MUL, op1=ADD)
                            qp = pa.tile([128, 512], f32, tag=f"qpart{j}",
                                         name=f"qpart{j}", bufs=2)
                            nc.vector.tensor_scalar(
                                out=qp[:], in0=qt[:, jl:jl + 512],
                                scalar1=meta[:, 2:3],
                                scalar2=segq_eff[:, db:db + 1],
                                op0=MUL, op1=ADD)
                            cparts.append(cp)
                            qparts.append(qp)
                        for j in range(2):
                            jl = j * 512
                            # bwd (sel=0) wants time-flipped q in half0 and
                            # time-flipped c in half1: block 1-j reversed
                            crev = pa.tile([128, 512], f32, tag="crev",
                                           bufs=2)
                            nc.scalar.copy(crev[:], cparts[1 - j][:, ::-1])
                            qrev = pa.tile([128, 512], f32, tag="qrev",
                                           bufs=2)
                            nc.scalar.copy(qrev[:], qparts[1 - j][:, ::-1])
                            d0 = pa.tile([128, 512], f32, tag="d0", bufs=2)
                            nc.vector.tensor_tensor(
                                out=d0[:], in0=cparts[j][:], in1=qrev[:],
                                op=SUB)
                            s0 = pa.tile([128, 512], f32, tag="s0", bufs=2)
                            nc.vector.tensor_scalar(
                                out=s0[:], in0=d0[:], scalar1=sel[:, 0:1],
                                scalar2=None, op0=MUL)
                            nc.vector.tensor_tensor(
                                out=x_r[db][:, jl:jl + 512],
                                in0=qrev[:], in1=s0[:], op=ADD)
                            d1 = pa.tile([128, 512], f32, tag="d1", bufs=2)
                            nc.vector.tensor_tensor(
                                out=d1[:], in0=qparts[j][:], in1=crev[:],
                                op=SUB)
                            s1 = pa.tile([128, 512], f32, tag="s1", bufs=2)
                            nc.vector.tensor_scalar(
                                out=s1[:], in0=d1[:], scalar1=sel[:, 0:1],
                                scalar2=None, op0=MUL)
                            nc.vector.tensor_tensor(
                                out=x_r[db][:, Lc + jl:Lc + jl + 512],
                                in0=crev[:], in1=s1[:], op=ADD)

                # ---------- phases B/C/D ----------
                with (tc.tile_pool(name="pb", bufs=1) as pb,
                      tc.tile_pool(name="psxp", bufs=1, space="PSUM") as psxp):
                    xp_acc = [psxp.tile([96, 512], f32, tag=f"xp{tb}",
                                        name=f"xp{tb}") for tb in range(NT)]
                    for e in range(NE):
                        wt = pb.tile([128, NK * 128], f16, tag="winstream",
                                     bufs=2)
                        nc.gpsimd.dma_start(wt[:], Win_d[e, :, :])
                        upre = pb.tile([128, L + 3], f32, tag="upre", bufs=2)
                        nc.gpsimd.memset(upre[:, 0:3], 0.0)
                        for tb in range(NT):
                            acc = ps.tile([128, 512], f32, tag="pp")
                            for kb in range(NK):
                                nc.tensor.matmul(
                                    acc[:], wt[:, kb * 128:(kb + 1) * 128],
                                    x_r[kb][:, tb * 512:(tb + 1) * 512],
                                    start=(kb == 0), stop=(kb == NK - 1))
                            nc.scalar.copy(
                                upre[:, 3 + tb * 512: 3 + (tb + 1) * 512],
                                acc[:])
                        # causal depthwise conv: taps read aligned slices
                        cacc = pb.tile([128, L], f32, tag="cacc0", bufs=2)
                        nc.vector.tensor_scalar(
                            out=cacc[:], in0=upre[:, 0:L],
                            scalar1=convw[:, e * DCONV: e * DCONV + 1],
                            scalar2=None, op0=MUL)
                        for k in (1, 2, 3):
                            nxt = pb.tile([128, L], f32, tag=f"cacc{k % 2}",
                                          name=f"cacc_{k}", bufs=2)
                            nc.vector.scalar_tensor_tensor(
                                out=nxt[:], in0=upre[:, k:k + L],
                                scalar=convw[:, e * DCONV + k:
                                             e * DCONV + k + 1],
                                in1=cacc[:], op0=MUL, op1=ADD)
                            cacc = nxt
                        usilu = pb.tile([128, L], f16, tag="usilu", bufs=2)
                        nc.scalar.activation(usilu[:], cacc[:], AF.Silu,
                                             bias=convb[:, e:e + 1])
                        nc.gpsimd.dma_start(
                            u_sp[e * 128:(e + 1) * 128, :], usilu[:])
                        for tb in range(NT):
                            nc.tensor.matmul(
                                xp_acc[tb][:],
                                Wxp[:, e * 96:(e + 1) * 96],
                                usilu[:, tb * 512:(tb + 1) * 512],
                                start=(e == 0), stop=(e == NE - 1))

                    # phase C: z half -> silu -> spill
                    for e in range(NE):
                        wt = pb.tile([128, NK * 128], f16, tag="winstream",
                                     name="wtz", bufs=2)
                        nc.gpsimd.dma_start(wt[:], Win_d[NE + e, :, :])
                        for tb in range(NT):
                            acc = ps.tile([128, 512], f32, tag="pp")
                            for kb in range(NK):
                                nc.tensor.matmul(
                                    acc[:], wt[:, kb * 128:(kb + 1) * 128],
                                    x_r[kb][:, tb * 512:(tb + 1) * 512],
                                    start=(kb == 0), stop=(kb == NK - 1))
                            zt = pb.tile([128, 512], bf16, tag="zt", bufs=2)
                            nc.scalar.activation(zt[:], acc[:], AF.Silu)
                            nc.sync.dma_start(
                                zs_sp[e * 128:(e + 1) * 128,
                                      tb * 512:(tb + 1) * 512], zt[:])

                    # phase D: x_proj epilogue
                    for tb in range(NT):
                        nc.scalar.copy(dt_r[:, tb * 512:(tb + 1) * 512],
                                       xp_acc[tb][0:DTR, :])
                        bct = pb.tile([2 * DS, 512], bf16, tag="bct", bufs=2)
                        nc.scalar.copy(bct[:], xp_acc[tb][DTR:96, :])
                        nc.sync.dma_start(
                            bc_sp[:, tb * 512:(tb + 1) * 512], bct[:])

            # ---------- phase E: dt_proj -> delta, dg ----------
            with tc.tile_pool(name="pe", bufs=1) as pe:
                for e in range(NE):
                    delta = pe.tile([128, L], f32, tag="delta", bufs=2)
                    for tb in range(NT):
                        acc = ps.tile([128, 512], f32, tag="pp")
                        nc.tensor.matmul(
                            acc[:], Wdt[:, e * 128:(e + 1) * 128],
                            dt_r[:, tb * 512:(tb + 1) * 512],
                            start=True, stop=True)
                        # softplus(x + b) = ln(1 + exp(x + b)); inputs here
                        # are small (|x|<6) so exp cannot overflow
                        ex = pe.tile([128, 512], f32, tag="spexp", bufs=2)
                        nc.scalar.activation(
                            ex[:], acc[:], AF.Exp, bias=dtb[:, e:e + 1])
                        nc.scalar.activation(
                            delta[:, tb * 512:(tb + 1) * 512], ex[:],
                            AF.Ln, bias=1.0)
                    nc.gpsimd.dma_start(
                        dl_sp[e * 128:(e + 1) * 128, :], delta[:])
                    ub = pe.tile([128, L], f16, tag="ub_e", bufs=2)
                    nc.sync.dma_start(ub[:], u_sp[e * 128:(e + 1) * 128, :])
                    dg = pe.tile([128, L], f16, tag="dg_e", bufs=2)
                    nc.vector.tensor_tensor(out=dg[:], in0=delta[:],
                                            in1=ub[:], op=MUL)
                    nc.sync.dma_start(
                        dg_sp[e * 128:(e + 1) * 128, :], dg[:])

            # ---------- phase F: selective scan ----------
            with tc.tile_pool(name="pf", bufs=1) as pf:
                for p in range(2):
                    Bb, Cb = [], []
                    for si in range(8):
                        s = p * 8 + si
                        bb = pf.tile([128, L], bf16, tag=f"Bb{si}",
                                     name=f"Bb{si}")
                        nc.sync.dma_start(
                            bb[:], bc_sp[s:s + 1, :].partition_broadcast(128))
                        cb = pf.tile([128, L], bf16, tag=f"Cb{si}",
                                     name=f"Cb{si}")
                        nc.sync.dma_start(
                            cb[:],
                            bc_sp[DS + s:DS + s + 1, :].partition_broadcast(128))
                        Bb.append(bb)
                        Cb.append(cb)
                    for e in range(NE):
                        dl = pf.tile([128, L], f16, tag="dl_f", bufs=2)
                        nc.sync.dma_start(
                            dl[:], dl_sp[e * 128:(e + 1) * 128, :])
                        dgt = pf.tile([128, L], f16, tag="dg_f", bufs=2)
                        nc.sync.dma_start(
                            dgt[:], dg_sp[e * 128:(e + 1) * 128, :])
                        if p == 0:
                            ub = pf.tile([128, L], f16, tag="ub_f", bufs=2)
                            nc.sync.dma_start(
                                ub[:], u_sp[e * 128:(e + 1) * 128, :])
                            yacc = pf.tile([128, L], f32, tag="yacc0",
                                           name="yacc_i", bufs=1)
                            nc.vector.tensor_scalar(
                                out=yacc[:], in0=ub[:],
                                scalar1=Dh[:, e:e + 1], scalar2=None, op0=MUL)
                        else:
                            yacc = pf.tile([128, L], f32, tag="yacc0",
                                           name="yacc_l", bufs=1)
                            nc.sync.dma_start(
                                yacc[:], yacc_sp[e * 128:(e + 1) * 128, :])
                        for si in range(8):
                            s = p * 8 + si
                            dA = pf.tile([128, L], f32, tag="dA", bufs=2)
                            nc.scalar.activation(
                                dA[:], dl[:], AF.Exp,
                                scale=Ah[:, e * DS + s: e * DS + s + 1])
                            dgB = pf.tile([128, L], bf16, tag="dgB", bufs=2)
                            nc.vector.tensor_tensor(
                                out=dgB[:], in0=dgt[:], in1=Bb[si][:], op=MUL)
                            h = pf.tile([128, L], bf16, tag="h", bufs=2)
                            nc.vector.tensor_tensor_scan(
                                h[:], dA[:], dgB[:], 0.0, op0=MUL, op1=ADD)
                            ch = pf.tile([128, L], bf16, tag="ch", bufs=2)
                            nc.vector.tensor_tensor(
                                out=ch[:], in0=h[:], in1=Cb[si][:], op=MUL)
                            ynew = pf.tile([128, L], f32,
                                           tag=f"yacc{(si + 1) % 2}",
                                           name=f"yacc_{si}", bufs=1)
                            nc.gpsimd.tensor_tensor(
                                out=ynew[:], in0=yacc[:], in1=ch[:], op=ADD)
                            yacc = ynew
                        if p == 0:
                            nc.sync.dma_start(
                                yacc_sp[e * 128:(e + 1) * 128, :], yacc[:])
                        else:
                            zst = pf.tile([128, L], bf16, tag="zs_f", bufs=2)
                            nc.sync.dma_start(
                                zst[:], zs_sp[e * 128:(e + 1) * 128, :])
                            yg = pf.tile([128, L], f16, tag="yg", bufs=2)
                            nc.vector.tensor_tensor(
                                out=yg[:], in0=yacc[:], in1=zst[:], op=MUL)
                            nc.sync.dma_start(
                                yg_sp[e * 128:(e + 1) * 128, :], yg[:])

            # ---------- phase G: out_proj on the sel-chosen half ----------
            # out_proj runs operand-swapped so og is TIME-major:
            # og[t, d] = sum_di ysel[di, t] * Wout[d, di]
            with tc.tile_pool(name="pg", bufs=1) as pg:
                Wout = pg.tile([128, NE * DM], f16, tag="Wout")
                nc.gpsimd.dma_start(Wout[:], Wout_d[:])
                ysel = [pg.tile([128, Lq], f16, tag=f"ys{kb}",
                                name=f"ys{kb}", bufs=1) for kb in range(NE)]
                for j in range(2):
                    jl = j * 512
                    for kb in range(NE):
                        # fwd (sel=1): natural cols Lc+jl..; bwd (sel=0):
                        # cols (1-j)*512.. time-reversed
                        ylo = pg.tile([128, 512], f16, tag="ylo", bufs=2)
                        nc.sync.dma_start(
                            ylo[:], yg_sp[kb * 128:(kb + 1) * 128,
                                          (1 - j) * 512:(2 - j) * 512])
                        yhi = pg.tile([128, 512], f16, tag="yhi", bufs=2)
                        nc.sync.dma_start(
                            yhi[:], yg_sp[kb * 128:(kb + 1) * 128,
                                          Lc + jl:Lc + jl + 512])
                        yrev = pg.tile([128, 512], f16, tag="yrev", bufs=2)
                        nc.scalar.copy(yrev[:], ylo[:, ::-1])
                        dft = pg.tile([128, 512], f32, tag="dft", bufs=2)
                        nc.vector.tensor_tensor(
                            out=dft[:], in0=yhi[:], in1=yrev[:], op=SUB)
                        sdf = pg.tile([128, 512], f32, tag="sdf", bufs=2)
                        nc.vector.tensor_scalar(
                            out=sdf[:], in0=dft[:], scalar1=sel[:, 0:1],
                            scalar2=None, op0=MUL)
                        nc.vector.tensor_tensor(
                            out=ysel[kb][:, jl:jl + 512], in0=yrev[:],
                            in1=sdf[:], op=ADD)
                for tb in range(8):
                    for dh in range(2):
                        acc = ps.tile([128, 512], f32, tag="pp")
                        for kb in range(NE):
                            nc.tensor.matmul(
                                acc[:],
                                ysel[kb][:, tb * 128:(tb + 1) * 128],
                                Wout[:, kb * DM + dh * 512:
                                     kb * DM + dh * 512 + 512],
                                start=(kb == 0), stop=(kb == NE - 1))
                        ot = pg.tile([128, 512], f16, tag="ot", bufs=2)
                        nc.scalar.copy(ot[:], acc[:])
                        nc.sync.dma_start(
                            og_sp[tb * 128:(tb + 1) * 128,
                                  dh * 512:dh * 512 + 512], ot[:])

                # pair AllReduce: fwd+bwd sum lands on both cores
                nc.gpsimd.collective_compute(
                    "AllReduce", ADD, replica_groups=PAIRS,
                    ins=[og_sp[:].opt()], outs=[og_sum[:].opt()])

                # each core outputs its dm-half (fwd cols 0:512, bwd
                # 512:1024), int8-quantized with a per-time-row f32
                # scale (scale includes the 0.5 fwd/bwd average factor)
                for tb in range(8):
                    r0, r1 = tb * 128, (tb + 1) * 128
                    stp = pg.tile([128, 512], f16, tag="stp", bufs=2)
                    nc.sync.dma_start(stp[:], og_sum[r0:r1, 0:512])
                    sbt = pg.tile([128, 512], f16, tag="sbt", bufs=2)
                    nc.sync.dma_start(sbt[:], og_sum[r0:r1, 512:1024])
                    dfo = pg.tile([128, 512], f32, tag="dfo", bufs=2)
                    nc.vector.tensor_tensor(
                        out=dfo[:], in0=stp[:], in1=sbt[:], op=SUB)
                    sfo = pg.tile([128, 512], f32, tag="sfo", bufs=2)
                    nc.vector.tensor_scalar(
                        out=sfo[:], in0=dfo[:], scalar1=sel[:, 0:1],
                        scalar2=None, op0=MUL)
                    oh = pg.tile([128, 512], f32, tag="oh", bufs=2)
                    nc.vector.tensor_tensor(
                        out=oh[:], in0=sbt[:], in1=sfo[:], op=ADD)
                    rmax = pg.tile([128, 1], f32, tag="rmax", bufs=2)
                    nc.vector.tensor_reduce(
                        out=rmax[:], in_=oh[:], axis=AX.X, op=MAXO,
                        apply_absolute_value=True)
                    srow = pg.tile([128, 1], f32, tag="srow", bufs=2)
                    nc.vector.tensor_scalar(
                        out=srow[:], in0=rmax[:], scalar1=1e-30,
                        scalar2=1.0 / 127, op0=MAXO, op1=MUL)
                    rinv = pg.tile([128, 1], f32, tag="rinv", bufs=2)
                    nc.vector.reciprocal(rinv[:], srow[:])
                    sdl = pg.tile([128, 1], f32, tag="sdl", bufs=2)
                    nc.vector.tensor_scalar(
                        out=sdl[:], in0=srow[:], scalar1=0.5,
                        scalar2=None, op0=MUL)
                    q8 = pg.tile([128, 512], i8, tag="q8", bufs=2)
                    nc.scalar.activation(q8[:], oh[:], AF.Copy,
                                         scale=rinv[:, 0:1])
                    nc.sync.dma_start(out_d[r0:r1, 0:512],
                                      q8[:].bitcast(u8))
                    nc.sync.dma_start(out_d[r0:r1, 512:516],
                                      sdl[:].bitcast(u8))

    nc.compile()
    return nc


def _install_cc_cache():
    """Content-keyed disk cache around the neuronx compiler hook.

    The bass_exec compile path (walrus) takes ~60 s for this program and
    has no persistent cache of its own; the emitted BIR (and hence the
    HLO carrying it) is byte-deterministic, so a sha256-of-HLO keyed
    cache makes every process after the first skip the compile.
    """
    import os
    try:
        import libneuronxla
    except ImportError:
        return
    if getattr(libneuronxla, "_bass_cc_disk_cache", False):
        return
    inner = libneuronxla.neuronx_cc
    cache_dir = os.environ.get(
        "NEURON_COMPILE_CACHE_URL",
        os.path.join(os.path.expanduser("~"), ".neuron-compile-cache"))
    try:
        os.makedirs(cache_dir, exist_ok=True)
    except OSError:
        libneuronxla._bass_cc_disk_cache = True
        return

    def cached(code, code_format, platform_version, file_prefix,
               *a, **kw):
        c = code if isinstance(code, (bytes, bytearray)) else \
            str(code).encode()
        key = hashlib.sha256(
            c + b"|" + str(platform_version).encode()).hexdigest()
        path = os.path.join(cache_dir, f"bassneff-{key}.hlo")
        try:
            with open(path, "rb") as f:
                return 0, f.read()
        except OSError:
            pass
        r = inner(code, code_format, platform_version, file_prefix,
                  *a, **kw)
        try:
            err, blob = r
            if err == 0 and isinstance(blob, (bytes, bytearray)) and blob:
                tmp = f"{path}.tmp.{os.getpid()}"
                with open(tmp, "wb") as f:
                    f.write(blob)
                os.replace(tmp, path)
        except Exception:
            pass
        return r

    libneuronxla.neuronx_cc = cached
    libneuronxla._bass_cc_disk_cache = True


def _make_runner(nc):
    """Jit the SPMD dispatch once; repeat calls hit the C++ fast path.

    Mirrors what bass_utils.run_bass_kernel_spmd does under axon
    (bass2jax.run_bass_via_pjrt), minus the per-call re-jit and the
    donated zero output buffers (the kernel writes every output
    element, so uninitialized outputs are fine).
    """
    import jax
    import numpy as np
    from jax.sharding import Mesh, PartitionSpec
    from jax.experimental.shard_map import shard_map
    from concourse import mybir
    from concourse.bass2jax import (_bass_exec_p, install_neuronx_cc_hook,
                                    partition_id_tensor)

    install_neuronx_cc_hook()
    _install_cc_cache()
    partition_name = (nc.partition_id_tensor.name
                      if nc.partition_id_tensor else None)
    in_names, out_names, out_avals = [], [], []
    for alloc in nc.m.functions[0].allocations:
        if not isinstance(alloc, mybir.MemoryLocationSet):
            continue
        name = alloc.memorylocations[0].name
        if alloc.kind == "ExternalInput":
            if name != partition_name:
                in_names.append(name)
        elif alloc.kind == "ExternalOutput":
            out_names.append(name)
            out_avals.append(jax.core.ShapedArray(
                tuple(alloc.tensor_shape), mybir.dt.np(alloc.dtype)))
    in_names_all = list(in_names)
    if partition_name is not None:
        in_names_all.append(partition_name)

    def _body(*args):
        operands = list(args)
        if partition_name is not None:
            operands.append(partition_id_tensor())
        return tuple(_bass_exec_p.bind(
            *operands, out_avals=tuple(out_avals),
            in_names=tuple(in_names_all), out_names=tuple(out_names),
            lowering_input_output_aliases=(),
            sim_require_finite=True, sim_require_nnan=True, nc=nc))

    devices = jax.devices()[:NCORE]
    mesh = Mesh(np.asarray(devices), ("core",))
    n_in = len(in_names)
    sharded = jax.jit(shard_map(
        _body, mesh=mesh, in_specs=(PartitionSpec("core"),) * n_in,
        out_specs=(PartitionSpec("core"),) * len(out_names),
        check_rep=False))

    def run(per_name_globals):
        args = [per_name_globals[name] for name in in_names]
        outs = sharded(*args)
        return {name: outs[i] for i, name in enumerate(out_names)}

    return run


_NF = NA * 128
_BUF = None
_MESH = None


def _mesh():
    global _MESH
    if _MESH is None:
        import jax
        from jax.sharding import Mesh, PartitionSpec, NamedSharding
        devs = jax.devices()[:NCORE]
        m = Mesh(np.asarray(devs), ("core",))
        _MESH = (m, devs, NamedSharding(m, PartitionSpec("core")))
    return _MESH


_NW = _NF + 128


def _bufs():
    global _BUF
    if _BUF is None:
        _BUF = dict(
            gcmb=np.zeros((NCORE, 513, _NW), np.uint8),
            u16=np.empty((512, DQ), np.uint16),
            f32=np.empty((512, _NF), np.float32),
            y=[np.zeros((B, Lq, DM), np.float32) for _ in range(2)],
            yi=0,
        )
    return _BUF


def kernel(**inputs) -> np.ndarray:
    global _RUN, _WFP
    bufs = _bufs()
    qf = np.asarray(inputs["query"], np.float32)
    cf_ = np.asarray(inputs["context"], np.float32)
    # ctx: 8-bit code u = round(x*s_c)+128; q: 9-bit u = round(x*s_q)+256
    # all host passes run on contiguous views (single CPU core)
    s_q = 255.0 / max(np.abs(qf).max(), 1e-30)
    s_c = 127.0 / max(np.abs(cf_).max(), 1e-30)
    gcmb = bufs["gcmb"]
    u16, ftmp = bufs["u16"], bufs["f32"]
    meta = np.zeros((NCORE, 4), np.float32)
    meta[:4, 0] = 1.0
    meta[:, 1] = 1.0 / s_c
    meta[:, 2] = 1.0 / s_q
    mbytes = meta.view(np.uint8).reshape(NCORE, 16)
    # device_put per core as soon as it is packed: puts are async, so
    # the tunnel upload streams while the host packs the next core
    import jax
    devs, shards = _mesh()[1], []
    for core in range(NCORE):
        b, hf = core % 4, core // 4
        hs = slice(hf * 512, (hf + 1) * 512)
        g = gcmb[core]
        np.multiply(cf_[b, hs], s_c, out=ftmp[:, :DC])
        np.add(ftmp[:, :DC], 128.5, out=g[:512, :DC], casting="unsafe")
        np.multiply(qf[b, hs], s_q, out=ftmp[:, DC:])
        np.add(ftmp[:, DC:], 256.5, out=u16, casting="unsafe")
        np.bitwise_and(u16, 255, out=g[:512, DC:_NF], casting="unsafe")
        hb = (u16 >> 8).astype(np.uint8).reshape(512, 128, 8)
        g[:512, _NF:] = np.packbits(hb, axis=2, bitorder="little")[:, :, 0]
        g[512, :16] = mbytes[core]
        shards.append(jax.device_put(g, devs[core]))

    # fingerprint the weights while the upload streams
    fp = _fingerprint(inputs)
    if _RUN is None or fp != _WFP:
        _RUN = _make_runner(_build(_prep_weights(inputs)))
        _WFP = fp

    ga = jax.make_array_from_single_device_arrays(
        (NCORE * 513, _NW), _mesh()[2], shards)
    res = _RUN({"acmb": ga})
    o = np.asarray(res["out"]).reshape(NCORE, Lq, DM // 2 + 4)
    q8 = o[:, :, :DM // 2].view(np.int8)
    sc = np.ascontiguousarray(o[:, :, DM // 2:]).view(np.float32)
    bufs["yi"] ^= 1
    y = bufs["y"][bufs["yi"]]
    for b in range(B):
        np.multiply(q8[b], sc[b], out=y[b, :, :DM // 2])
        np.multiply(q8[4 + b], sc[4 + b], out=y[b, :, DM // 2:])
    return y



# revision 11
# speedup vs baseline: 1.1758x; 1.0815x over previous
"""CrossMamba Trainium2 kernel.

Sharding: 8 cores = 4 batches x 2 scan directions. Core b and core 4+b
form a pair that works on batch b; both run the same SPMD program and
differ only in a 4-byte selector in the meta row (sel=1 fwd, 0 bwd).

Wall-clock is dominated by the axon tunnel (~30-100 MB/s shared, ~80 ms
fixed latency per direction; device exec is only ~3.6 ms), so the I/O
contract is tuned for wire bytes and host (single-CPU) cost:
  - all weights are baked into the NEFF as inline Const tensors
    (transferred once at executable load, never per call)
  - activations upload quantized: ctx as an 8-bit code (its noise is
    averaged down by the 768-wide c_in GEMM), q as a 9-bit code
    (low byte + bit-packed high bit). 0.94 MB per core, 7.5 MB total.
    Codes are packed host-side in natural [time, feature] layout with
    contiguous numpy ops only; the device unpacks (int shift/and ops)
    and PE-transposes to feature-major. The +2^(b-1) code bias is
    folded into effective seg biases (for ctx via colsum(c_in_w)), so
    dequantization costs no extra per-element work
  - each core uploads only HALF of its batch's sequence; an on-device
    pair AllGather (over f32-viewed byte buffers) reassembles it
  - per-core shards are device_put as soon as they are packed (puts
    are async), overlapping host packing with the upload stream; the
    output is fetched without an intermediate block_until_ready, which
    hides the execute round trip
  - the backward direction is derived on device: exact 0/1 sel-blends
    choose operand placement, and negative-stride (reversed-AP) copies
    time-flip the data, so fwd and bwd cores run one program
  - out_proj runs operand-swapped so the output is TIME-major, the
    fwd+bwd results are summed with a pair AllReduce, and each core
    downloads its dm-half int8-quantized with a per-time-row f32 scale
    (scale carries the 0.5 fwd/bwd average): 0.53 MB per core, 4.3 MB
    total. Host dequant is two contiguous broadcasts per batch
  - the jitted executable is cached at module level keyed on a weight
    fingerprint; repeat calls skip re-trace/re-compile

Per-core program:
  A0) bounce upload to DRAM, pair AllGather -> full time range
  A) unpack codes, PE-transpose, x = blend(c_in(ctx)+segc', q+segq')
     with sel-driven placement/flip
  B) in_proj (u half) -> causal depthwise conv -> silu -> x_proj acc
  C) in_proj (z half) -> silu -> spill
  D) x_proj epilogue (dt / B / C rows)
  E) dt_proj -> softplus -> delta, dg = delta*u
  F) selective scan: per (channel-block, state): dA = exp(A_s*delta),
     dgB, hardware tensor_tensor_scan, C-readout, state accumulation;
     two passes of 8 states
  G) gate with silu(z), sel-chosen/flipped time half -> time-major
     out_proj, pair AllReduce, int8 quantize, output the dm half

GEMMs run in fp16 (f32 PSUM accumulate), scan math in f32/bf16.
End-to-end relative error vs the fp32 reference: ~9e-3 (gate: 2e-2).
"""
import hashlib
import numpy as np

B, Lq, Lc = 4, 1024, 1024
DQ, DC, DM = 1024, 768, 1024
DS, DCONV = 16, 4
DI, DTR = 2048, 64
L = Lc + Lq              # 2048
NCORE = 8
NE = DI // 128           # 16 u (or z) channel blocks
NK = DM // 128           # 8 k blocks for in_proj
NT = L // 512            # 4 time blocks of 512
NA = (DC + DM) // 128    # 14 row blocks in the packed activation half

_RUN = None              # cached (runner, weight fingerprint)
_WFP = None

_WKEYS = ("c_in_w", "seg_context", "seg_query", "in_proj_w", "conv_w",
          "conv_b", "x_proj_w", "dt_proj_w", "dt_proj_b", "A_log", "D",
          "mamba_out_w")


def _fingerprint(inputs):
    h = hashlib.blake2b(digest_size=16)
    for k in _WKEYS:
        a = np.ascontiguousarray(np.asarray(inputs[k]))
        h.update(k.encode())
        h.update(str(a.shape).encode())
        b = a.view(np.uint8).reshape(-1)
        step = max(1, b.size // 65536)
        h.update(bytes(b[::step][:65536]))
    return h.digest()


def _prep_weights(inputs):
    f32, f16 = np.float32, np.float16
    c_in_w = np.asarray(inputs["c_in_w"], f32)
    segc = np.asarray(inputs["seg_context"], f32).reshape(DM)
    segq = np.asarray(inputs["seg_query"], f32).reshape(DM)
    in_proj_w = np.asarray(inputs["in_proj_w"], f32)
    conv_w = np.asarray(inputs["conv_w"], f32)
    conv_b = np.asarray(inputs["conv_b"], f32)
    x_proj_w = np.asarray(inputs["x_proj_w"], f32)
    dt_proj_w = np.asarray(inputs["dt_proj_w"], f32)
    dt_proj_b = np.asarray(inputs["dt_proj_b"], f32)
    A = (-np.exp(np.asarray(inputs["A_log"], f32))).astype(f32)
    D = np.asarray(inputs["D"], f32)
    out_w = np.asarray(inputs["mamba_out_w"], f32)

    def blk(a, p=128):
        # [n*p, m] -> [p, n*m] with n-major free layout
        n = a.shape[0] // p
        return np.ascontiguousarray(
            a.reshape(n, p, -1).transpose(1, 0, 2).reshape(p, -1))

    return dict(
        Wc=blk(c_in_w.T).astype(f16),                     # [128, 6*1024]
        segc=np.ascontiguousarray(segc.reshape(NK, 128).T),   # [128, 8]
        segq=np.ascontiguousarray(segq.reshape(NK, 128).T),
        # row sums of c_in_w (over the DC axis): used to fold the
        # +512 bias of the 10-bit activation code out of the c_in GEMM
        csum=np.ascontiguousarray(
            c_in_w.sum(axis=1).reshape(NK, 128).T),       # [128, 8]
        Win=np.ascontiguousarray(
            in_proj_w.reshape(32, 128, NK, 128).transpose(0, 3, 2, 1)
            .reshape(32, 128, NK * 128)).astype(f16),     # [32,128,1024]
        Wxp=blk(x_proj_w.T).astype(f16),                  # [128, 16*96]
        Wdt=np.ascontiguousarray(dt_proj_w.T).astype(f16),  # [64, 2048]
        Wout=np.ascontiguousarray(
            out_w.reshape(8, 128, NE, 128).transpose(3, 2, 0, 1)
            .reshape(128, NE * DM)).astype(f16),          # [128, 16*1024]
        convw=blk(conv_w),                                # [128, 16*4]
        convb=conv_b.reshape(NE, 128).T.copy(),
        dtb=dt_proj_b.reshape(NE, 128).T.copy(),
        Ah=blk(A),                                        # [128, 16*16]
        Dh=D.reshape(NE, 128).T.copy(),
    )


def _build(w):
    import concourse.bacc as bacc
    import concourse.tile as tile
    from concourse import mybir

    f32 = mybir.dt.float32
    f16 = mybir.dt.float16
    bf16 = mybir.dt.bfloat16
    u8 = mybir.dt.uint8
    i8 = mybir.dt.int8
    i32 = mybir.dt.int32
    MUL = mybir.AluOpType.mult
    ADD = mybir.AluOpType.add
    SUB = mybir.AluOpType.subtract
    BYP = mybir.AluOpType.bypass
    MAXO = mybir.AluOpType.max
    SHR = mybir.AluOpType.logical_shift_right
    AND = mybir.AluOpType.bitwise_and
    AF = mybir.ActivationFunctionType
    AX = mybir.AxisListType
    PAIRS = [[0, 4], [1, 5], [2, 6], [3, 7]]

    nc = bacc.Bacc("TRN2", target_bir_lowering=False, debug=False,
                   num_devices=NCORE)

    # ---- per-core external inputs ----
    # Quantized activations for this core's half of the batch's [ctx, q]
    # feature-concat, NATURAL [time, feature] layout (host packs with
    # contiguous ops only; the device transposes). Core b carries times
    # 0:512, core 4+b times 512:1024.
    # ctx uses an 8-bit code u = round(x*s_c)+128 (its quantization
    # noise is averaged down by the 768-wide c_in GEMM); q uses a 9-bit
    # code u = round(x*s_q)+256 (it enters x directly).
    # One combined tensor per core (one put, one AllGather):
    #   rows 0:512: ctx code bytes (cols 0:768) | q low bytes (cols
    #     768:1792) | q high bits, 8 consecutive features per byte,
    #     little bit order (cols 1792:1920)
    #   row 512: 16 meta bytes = f32 [sel, 1/s_ctx, 1/s_q, 0]
    NF = NA * 128            # 1792 features (ctx 768 | q 1024)
    NW = NF + 128            # +128 hi-bit bytes
    cmb_d = nc.dram_tensor("acmb", [513, NW], u8, kind="ExternalInput")

    # ---- weights baked into the NEFF (loaded once, not per call) ----
    Wc_d = nc.inline_tensor(w["Wc"], name="Wc_i")
    segc_d = nc.inline_tensor(w["segc"], name="segc_i")
    segq_d = nc.inline_tensor(w["segq"], name="segq_i")
    csum_d = nc.inline_tensor(w["csum"], name="csum_i")
    Win_d = nc.inline_tensor(w["Win"], name="Win_i")
    Wxp_d = nc.inline_tensor(w["Wxp"], name="Wxp_i")
    Wdt_d = nc.inline_tensor(w["Wdt"], name="Wdt_i")
    Wout_d = nc.inline_tensor(w["Wout"], name="Wout_i")
    convw_d = nc.inline_tensor(w["convw"], name="convw_i")
    convb_d = nc.inline_tensor(w["convb"], name="convb_i")
    dtb_d = nc.inline_tensor(w["dtb"], name="dtb_i")
    Ah_d = nc.inline_tensor(w["Ah"], name="Ah_i")
    Dh_d = nc.inline_tensor(w["Dh"], name="Dh_i")
    eye_d = nc.inline_tensor(np.eye(128, dtype=np.float16), name="eye_i")

    # ---- DRAM scratch ----
    # bounce/gather buffers are declared f32 (collective-safe dtype) and
    # byte-addressed via bitcast; widths are bytes/4
    cmb_bnc = nc.dram_tensor("cmb_bnc", [512, NW // 4], f32)
    ag_cmb = nc.dram_tensor("ag_cmb", [2, 512, NW // 4], f32)
    u_sp = nc.dram_tensor("u_sp", [DI, L], f16)
    zs_sp = nc.dram_tensor("zs_sp", [DI, L], bf16)
    dl_sp = nc.dram_tensor("dl_sp", [DI, L], f16)
    dg_sp = nc.dram_tensor("dg_sp", [DI, L], f16)
    bc_sp = nc.dram_tensor("bc_sp", [2 * DS, L], bf16)
    yacc_sp = nc.dram_tensor("yacc_sp", [DI, L], f32)
    yg_sp = nc.dram_tensor("yg_sp", [DI, L], f16)
    og_sp = nc.dram_tensor("og_sp", [Lq, DM], f16)
    og_sum = nc.dram_tensor("og_sum", [Lq, DM], f16)

    # time-major int8 output + per-time-row f32 scale in cols 512:516
    out_d = nc.dram_tensor("out", [Lq, DM // 2 + 4], u8,
                           kind="ExternalOutput")

    with tile.TileContext(nc) as tc:
        with (
            tc.tile_pool(name="wp", bufs=1) as wp,
            tc.tile_pool(name="ps", bufs=3, space="PSUM") as ps,
        ):
            # ---------- phase A0: bounce + pair AllGather ----------
            with tc.tile_pool(name="p0", bufs=2) as p0:
                for rb in range(4):
                    r0, r1 = rb * 128, (rb + 1) * 128
                    bt = p0.tile([128, NW], u8, tag="bnc")
                    nc.sync.dma_start(bt[:], cmb_d[r0:r1, :])
                    nc.sync.dma_start(cmb_bnc[r0:r1, :].bitcast(u8), bt[:])
            nc.gpsimd.collective_compute(
                "AllGather", BYP, replica_groups=PAIRS,
                ins=[cmb_bnc[:].opt()], outs=[ag_cmb[:].opt()])

            # ---------- small persistent weights ----------
            convw = wp.tile([128, NE * DCONV], f32, tag="convw")
            nc.sync.dma_start(convw[:], convw_d[:])
            convb = wp.tile([128, NE], f32, tag="convb")
            nc.sync.dma_start(convb[:], convb_d[:])
            dtb = wp.tile([128, NE], f32, tag="dtb")
            nc.sync.dma_start(dtb[:], dtb_d[:])
            Ah = wp.tile([128, NE * DS], f32, tag="Ah")
            nc.sync.dma_start(Ah[:], Ah_d[:])
            Dh = wp.tile([128, NE], f32, tag="Dh")
            nc.sync.dma_start(Dh[:], Dh_d[:])
            Wxp = wp.tile([128, NE * 96], f16, tag="Wxp")
            nc.gpsimd.dma_start(Wxp[:], Wxp_d[:])
            Wdt = wp.tile([DTR, DI], f16, tag="Wdt")
            nc.gpsimd.dma_start(Wdt[:], Wdt_d[:])
            dt_r = wp.tile([DTR, L], f16, tag="dt_r")
            # meta = [sel, inv_sc, inv_sq, 0] broadcast to all partitions
            meta = wp.tile([128, 4], f32, tag="meta")
            nc.sync.dma_start(
                meta[:], cmb_d[512:513, 0:16]
                .bitcast(f32).partition_broadcast(128))
            sel = meta
            ident = wp.tile([128, 128], f16, tag="ident")
            nc.sync.dma_start(ident[:], eye_d[:])

            with tc.tile_pool(name="px", bufs=1) as px:
                # full-sequence x, fp16, 32 KB/part; lives phases A-C
                x_r = [px.tile([128, L], f16, tag=f"x{db}", name=f"x{db}")
                       for db in range(NK)]

                # ---------- phase A ----------
                with (tc.tile_pool(name="pa", bufs=1) as pa,
                      tc.tile_pool(name="pst", bufs=2,
                                   space="PSUM") as pst):
                    Wc = pa.tile([128, 6 * DM], f16, tag="Wc")
                    nc.gpsimd.dma_start(Wc[:], Wc_d[:])
                    segc = pa.tile([128, NK], f32, tag="segc")
                    nc.sync.dma_start(segc[:], segc_d[:])
                    segq = pa.tile([128, NK], f32, tag="segq")
                    nc.sync.dma_start(segq[:], segq_d[:])
                    csum = pa.tile([128, NK], f32, tag="csum")
                    nc.sync.dma_start(csum[:], csum_d[:])
                    # fold the code biases into effective seg biases:
                    #   segc_eff = segc - 128*inv_sc*csum
                    #   segq_eff = segq - 256*inv_sq
                    m128c = pa.tile([128, 1], f32, tag="m128c")
                    nc.vector.tensor_scalar(
                        out=m128c[:], in0=meta[:, 1:2], scalar1=-128.0,
                        scalar2=None, op0=MUL)
                    m256q = pa.tile([128, 1], f32, tag="m256q")
                    nc.vector.tensor_scalar(
                        out=m256q[:], in0=meta[:, 2:3], scalar1=-256.0,
                        scalar2=None, op0=MUL)
                    segc_eff = pa.tile([128, NK], f32, tag="segc_eff")
                    nc.vector.scalar_tensor_tensor(
                        out=segc_eff[:], in0=csum[:],
                        scalar=m128c[:, 0:1], in1=segc[:],
                        op0=MUL, op1=ADD)
                    segq_eff = pa.tile([128, NK], f32, tag="segq_eff")
                    nc.vector.tensor_scalar(
                        out=segq_eff[:], in0=segq[:],
                        scalar1=m256q[:, 0:1], scalar2=None, op0=ADD)

                    # unpack the codes in their uploaded time-major
                    # layout, then PE-transpose into feature-major tiles
                    fm = [pa.tile([128, 1024], f16, tag=f"fm{fb}",
                                  name=f"fm{fb}", bufs=1)
                          for fb in range(NA)]
                    for tb in range(8):
                        hf, r0 = tb // 4, (tb % 4) * 128
                        lot = pa.tile([128, NF], u8, tag="lot", bufs=2)
                        nc.sync.dma_start(
                            lot[:],
                            ag_cmb[hf, r0:r0 + 128, 0:NF // 4].bitcast(u8))
                        hit = pa.tile([128, 128], u8, tag="hit", bufs=2)
                        nc.sync.dma_start(
                            hit[:],
                            ag_cmb[hf, r0:r0 + 128,
                                   NF // 4:NW // 4].bitcast(u8))
                        lof = pa.tile([128, NF], f16, tag="lof", bufs=2)
                        nc.scalar.copy(lof[:], lot[:])
                        hi32 = pa.tile([128, 128], i32, tag="hi32",
                                       bufs=2)
                        nc.scalar.copy(hi32[:], hit[:])
                        uq = pa.tile([128, 1024], f16, tag="uq", bufs=2)
                        for k in range(8):
                            hk = pa.tile([128, 128], i32, tag="hk",
                                         bufs=2)
                            nc.vector.tensor_scalar(
                                out=hk[:], in0=hi32[:], scalar1=k,
                                scalar2=1, op0=SHR, op1=AND)
                            hkf = pa.tile([128, 128], f16, tag="hkf",
                                          bufs=2)
                            nc.scalar.copy(hkf[:], hk[:])
                            nc.vector.scalar_tensor_tensor(
                                out=uq[:, k::8], in0=hkf[:], scalar=256.0,
                                in1=lof[:, DC + k::8], op0=MUL, op1=ADD)
                        for fb in range(NA):
                            src = (lof[:, fb * 128:(fb + 1) * 128]
                                   if fb < 6 else
                                   uq[:, (fb - 6) * 128:(fb - 5) * 128])
                            tp = pst.tile([128, 128], f16, tag="tp")
                            nc.tensor.transpose(tp[:], src, ident[:])
                            nc.scalar.copy(
                                fm[fb][:, tb * 128:(tb + 1) * 128], tp[:])
                    ctx_sb = fm[:6]
                    for db in range(NK):
                        qt = fm[6 + db]
                        cparts, qparts = [], []
                        for j in range(2):
                            jl = j * 512
                            acc = ps.tile([128, 512], f32, tag="pp")
                            for kb in range(6):
                                nc.tensor.matmul(
                                    acc[:],
                                    Wc[:, kb * DM + db * 128:
                                       kb * DM + (db + 1) * 128],
                                    ctx_sb[kb][:, jl:jl + 512],
                                    start=(kb == 0), stop=(kb == 5))
                            cp = pa.tile([128, 512], f32, tag=f"cpart{j}",
                                         name=f"cpart{j}", bufs=2)
                            nc.vector.tensor_scalar(
                                out=cp[:], in0=acc[:],
                                scalar1=meta[:, 1:2],
                                scalar2=segc_eff[:, db:db + 1],
                                op0=MUL, op1=ADD)
                            qp = pa.tile([128, 512], f32, tag=f"qpart{j}",
                                         name=f"qpart{j}", bufs=2)
                            nc.vector.tensor_scalar(
                                out=qp[:], in0=qt[:, jl:jl + 512],
                                scalar1=meta[:, 2:3],
                                scalar2=segq_eff[:, db:db + 1],
                                op0=MUL, op1=ADD)
                            cparts.append(cp)
                            qparts.append(qp)
                        for j in range(2):
                            jl = j * 512
                            # bwd (sel=0) wants time-flipped q in half0 and
                            # time-flipped c in half1: block 1-j reversed
                            crev = pa.tile([128, 512], f32, tag="crev",
                                           bufs=2)
                            nc.scalar.copy(crev[:], cparts[1 - j][:, ::-1])
                            qrev = pa.tile([128, 512], f32, tag="qrev",
                                           bufs=2)
                            nc.scalar.copy(qrev[:], qparts[1 - j][:, ::-1])
                            d0 = pa.tile([128, 512], f32, tag="d0", bufs=2)
                            nc.vector.tensor_tensor(
                                out=d0[:], in0=cparts[j][:], in1=qrev[:],
                                op=SUB)
                            s0 = pa.tile([128, 512], f32, tag="s0", bufs=2)
                            nc.vector.tensor_scalar(
                                out=s0[:], in0=d0[:], scalar1=sel[:, 0:1],
                                scalar2=None, op0=MUL)
                            nc.vector.tensor_tensor(
                                out=x_r[db][:, jl:jl + 512],
                                in0=qrev[:], in1=s0[:], op=ADD)
                            d1 = pa.tile([128, 512], f32, tag="d1", bufs=2)
                            nc.vector.tensor_tensor(
                                out=d1[:], in0=qparts[j][:], in1=crev[:],
                                op=SUB)
                            s1 = pa.tile([128, 512], f32, tag="s1", bufs=2)
                            nc.vector.tensor_scalar(
                                out=s1[:], in0=d1[:], scalar1=sel[:, 0:1],
                                scalar2=None, op0=MUL)
                            nc.vector.tensor_tensor(
                                out=x_r[db][:, Lc + jl:Lc + jl + 512],
                                in0=crev[:], in1=s1[:], op=ADD)

                # ---------- phases B/C/D ----------
                with (tc.tile_pool(name="pb", bufs=1) as pb,
                      tc.tile_pool(name="psxp", bufs=1, space="PSUM") as psxp):
                    xp_acc = [psxp.tile([96, 512], f32, tag=f"xp{tb}",
                                        name=f"xp{tb}") for tb in range(NT)]
                    for e in range(NE):
                        wt = pb.tile([128, NK * 128], f16, tag="winstream",
                                     bufs=2)
                        nc.gpsimd.dma_start(wt[:], Win_d[e, :, :])
                        upre = pb.tile([128, L + 3], f32, tag="upre", bufs=2)
                        nc.gpsimd.memset(upre[:, 0:3], 0.0)
                        for tb in range(NT):
                            acc = ps.tile([128, 512], f32, tag="pp")
                            for kb in range(NK):
                                nc.tensor.matmul(
                                    acc[:], wt[:, kb * 128:(kb + 1) * 128],
                                    x_r[kb][:, tb * 512:(tb + 1) * 512],
                                    start=(kb == 0), stop=(kb == NK - 1))
                            nc.scalar.copy(
                                upre[:, 3 + tb * 512: 3 + (tb + 1) * 512],
                                acc[:])
                        # causal depthwise conv: taps read aligned slices
                        cacc = pb.tile([128, L], f32, tag="cacc0", bufs=2)
                        nc.vector.tensor_scalar(
                            out=cacc[:], in0=upre[:, 0:L],
                            scalar1=convw[:, e * DCONV: e * DCONV + 1],
                            scalar2=None, op0=MUL)
                        for k in (1, 2, 3):
                            nxt = pb.tile([128, L], f32, tag=f"cacc{k % 2}",
                                          name=f"cacc_{k}", bufs=2)
                            nc.vector.scalar_tensor_tensor(
                                out=nxt[:], in0=upre[:, k:k + L],
                                scalar=convw[:, e * DCONV + k:
                                             e * DCONV + k + 1],
                                in1=cacc[:], op0=MUL, op1=ADD)
                            cacc = nxt
                        usilu = pb.tile([128, L], f16, tag="usilu", bufs=2)
                        nc.scalar.activation(usilu[:], cacc[:], AF.Silu,
                                             bias=convb[:, e:e + 1])
                        nc.gpsimd.dma_start(
                            u_sp[e * 128:(e + 1) * 128, :], usilu[:])
                        for tb in range(NT):
                            nc.tensor.matmul(
                                xp_acc[tb][:],
                                Wxp[:, e * 96:(e + 1) * 96],
                                usilu[:, tb * 512:(tb + 1) * 512],
                                start=(e == 0), stop=(e == NE - 1))

                    # phase C: z half -> silu -> spill
                    for e in range(NE):
                        wt = pb.tile([128, NK * 128], f16, tag="winstream",
                                     name="wtz", bufs=2)
                        nc.gpsimd.dma_start(wt[:], Win_d[NE + e, :, :])
                        for tb in range(NT):
                            acc = ps.tile([128, 512], f32, tag="pp")
                            for kb in range(NK):
                                nc.tensor.matmul(
                                    acc[:], wt[:, kb * 128:(kb + 1) * 128],
                                    x_r[kb][:, tb * 512:(tb + 1) * 512],
                                    start=(kb == 0), stop=(kb == NK - 1))
                            zt = pb.tile([128, 512], bf16, tag="zt", bufs=2)
                            nc.scalar.activation(zt[:], acc[:], AF.Silu)
                            nc.sync.dma_start(
                                zs_sp[e * 128:(e + 1) * 128,
                                      tb * 512:(tb + 1) * 512], zt[:])

                    # phase D: x_proj epilogue
                    for tb in range(NT):
                        nc.scalar.copy(dt_r[:, tb * 512:(tb + 1) * 512],
                                       xp_acc[tb][0:DTR, :])
                        bct = pb.tile([2 * DS, 512], bf16, tag="bct", bufs=2)
                        nc.scalar.copy(bct[:], xp_acc[tb][DTR:96, :])
                        nc.sync.dma_start(
                            bc_sp[:, tb * 512:(tb + 1) * 512], bct[:])

            # ---------- phase E: dt_proj -> delta, dg ----------
            with tc.tile_pool(name="pe", bufs=1) as pe:
                for e in range(NE):
                    delta = pe.tile([128, L], f32, tag="delta", bufs=2)
                    for tb in range(NT):
                        acc = ps.tile([128, 512], f32, tag="pp")
                        nc.tensor.matmul(
                            acc[:], Wdt[:, e * 128:(e + 1) * 128],
                            dt_r[:, tb * 512:(tb + 1) * 512],
                            start=True, stop=True)
                        # softplus(x + b) = ln(1 + exp(x + b)); inputs here
                        # are small (|x|<6) so exp cannot overflow
                        ex = pe.tile([128, 512], f32, tag="spexp", bufs=2)
                        nc.scalar.activation(
                            ex[:], acc[:], AF.Exp, bias=dtb[:, e:e + 1])
                        nc.scalar.activation(
                            delta[:, tb * 512:(tb + 1) * 512], ex[:],
                            AF.Ln, bias=1.0)
                    nc.gpsimd.dma_start(
                        dl_sp[e * 128:(e + 1) * 128, :], delta[:])
                    ub = pe.tile([128, L], f16, tag="ub_e", bufs=2)
                    nc.sync.dma_start(ub[:], u_sp[e * 128:(e + 1) * 128, :])
                    dg = pe.tile([128, L], f16, tag="dg_e", bufs=2)
                    nc.vector.tensor_tensor(out=dg[:], in0=delta[:],
                                            in1=ub[:], op=MUL)
                    nc.sync.dma_start(
                        dg_sp[e * 128:(e + 1) * 128, :], dg[:])

            # ---------- phase F: selective scan ----------
            with tc.tile_pool(name="pf", bufs=1) as pf:
                for p in range(2):
                    Bb, Cb = [], []
                    for si in range(8):
                        s = p * 8 + si
                        bb = pf.tile([128, L], bf16, tag=f"Bb{si}",
                                     name=f"Bb{si}")
                        nc.sync.dma_start(
                            bb[:], bc_sp[s:s + 1, :].partition_broadcast(128))
                        cb = pf.tile([128, L], bf16, tag=f"Cb{si}",
                                     name=f"Cb{si}")
                        nc.sync.dma_start(
                            cb[:],
                            bc_sp[DS + s:DS + s + 1, :].partition_broadcast(128))
                        Bb.append(bb)
                        Cb.append(cb)
                    for e in range(NE):
                        dl = pf.tile([128, L], f16, tag="dl_f", bufs=2)
                        nc.sync.dma_start(
                            dl[:], dl_sp[e * 128:(e + 1) * 128, :])
                        dgt = pf.tile([128, L], f16, tag="dg_f", bufs=2)
                        nc.sync.dma_start(
                            dgt[:], dg_sp[e * 128:(e + 1) * 128, :])
                        if p == 0:
                            ub = pf.tile([128, L], f16, tag="ub_f", bufs=2)
                            nc.sync.dma_start(
                                ub[:], u_sp[e * 128:(e + 1) * 128, :])
                            yacc = pf.tile([128, L], f32, tag="yacc0",
                                           name="yacc_i", bufs=1)
                            nc.vector.tensor_scalar(
                                out=yacc[:], in0=ub[:],
                                scalar1=Dh[:, e:e + 1], scalar2=None, op0=MUL)
                        else:
                            yacc = pf.tile([128, L], f32, tag="yacc0",
                                           name="yacc_l", bufs=1)
                            nc.sync.dma_start(
                                yacc[:], yacc_sp[e * 128:(e + 1) * 128, :])
                        for si in range(8):
                            s = p * 8 + si
                            dA = pf.tile([128, L], f32, tag="dA", bufs=2)
                            nc.scalar.activation(
                                dA[:], dl[:], AF.Exp,
                                scale=Ah[:, e * DS + s: e * DS + s + 1])
                            dgB = pf.tile([128, L], bf16, tag="dgB", bufs=2)
                            nc.vector.tensor_tensor(
                                out=dgB[:], in0=dgt[:], in1=Bb[si][:], op=MUL)
                            h = pf.tile([128, L], bf16, tag="h", bufs=2)
                            nc.vector.tensor_tensor_scan(
                                h[:], dA[:], dgB[:], 0.0, op0=MUL, op1=ADD)
                            ch = pf.tile([128, L], bf16, tag="ch", bufs=2)
                            nc.vector.tensor_tensor(
                                out=ch[:], in0=h[:], in1=Cb[si][:], op=MUL)
                            ynew = pf.tile([128, L], f32,
                                           tag=f"yacc{(si + 1) % 2}",
                                           name=f"yacc_{si}", bufs=1)
                            nc.gpsimd.tensor_tensor(
                                out=ynew[:], in0=yacc[:], in1=ch[:], op=ADD)
                            yacc = ynew
                        if p == 0:
                            nc.sync.dma_start(
                                yacc_sp[e * 128:(e + 1) * 128, :], yacc[:])
                        else:
                            zst = pf.tile([128, L], bf16, tag="zs_f", bufs=2)
                            nc.sync.dma_start(
                                zst[:], zs_sp[e * 128:(e + 1) * 128, :])
                            yg = pf.tile([128, L], f16, tag="yg", bufs=2)
                            nc.vector.tensor_tensor(
                                out=yg[:], in0=yacc[:], in1=zst[:], op=MUL)
                            nc.sync.dma_start(
                                yg_sp[e * 128:(e + 1) * 128, :], yg[:])

            # ---------- phase G: out_proj on the sel-chosen half ----------
            # out_proj runs operand-swapped so og is TIME-major:
            # og[t, d] = sum_di ysel[di, t] * Wout[d, di]
            with tc.tile_pool(name="pg", bufs=1) as pg:
                Wout = pg.tile([128, NE * DM], f16, tag="Wout")
                nc.gpsimd.dma_start(Wout[:], Wout_d[:])
                ysel = [pg.tile([128, Lq], f16, tag=f"ys{kb}",
                                name=f"ys{kb}", bufs=1) for kb in range(NE)]
                for j in range(2):
                    jl = j * 512
                    for kb in range(NE):
                        # fwd (sel=1): natural cols Lc+jl..; bwd (sel=0):
                        # cols (1-j)*512.. time-reversed
                        ylo = pg.tile([128, 512], f16, tag="ylo", bufs=2)
                        nc.sync.dma_start(
                            ylo[:], yg_sp[kb * 128:(kb + 1) * 128,
                                          (1 - j) * 512:(2 - j) * 512])
                        yhi = pg.tile([128, 512], f16, tag="yhi", bufs=2)
                        nc.sync.dma_start(
                            yhi[:], yg_sp[kb * 128:(kb + 1) * 128,
                                          Lc + jl:Lc + jl + 512])
                        yrev = pg.tile([128, 512], f16, tag="yrev", bufs=2)
                        nc.scalar.copy(yrev[:], ylo[:, ::-1])
                        dft = pg.tile([128, 512], f32, tag="dft", bufs=2)
                        nc.vector.tensor_tensor(
                            out=dft[:], in0=yhi[:], in1=yrev[:], op=SUB)
                        sdf = pg.tile([128, 512], f32, tag="sdf", bufs=2)
                        nc.vector.tensor_scalar(
                            out=sdf[:], in0=dft[:], scalar1=sel[:, 0:1],
                            scalar2=None, op0=MUL)
                        nc.vector.tensor_tensor(
                            out=ysel[kb][:, jl:jl + 512], in0=yrev[:],
                            in1=sdf[:], op=ADD)
                for tb in range(8):
                    for dh in range(2):
                        acc = ps.tile([128, 512], f32, tag="pp")
                        for kb in range(NE):
                            nc.tensor.matmul(
                                acc[:],
                                ysel[kb][:, tb * 128:(tb + 1) * 128],
                                Wout[:, kb * DM + dh * 512:
                                     kb * DM + dh * 512 + 512],
                                start=(kb == 0), stop=(kb == NE - 1))
                        ot = pg.tile([128, 512], f16, tag="ot", bufs=2)
                        nc.scalar.copy(ot[:], acc[:])
                        nc.sync.dma_start(
                            og_sp[tb * 128:(tb + 1) * 128,
                                  dh * 512:dh * 512 + 512], ot[:])

                # pair AllReduce: fwd+bwd sum lands on both cores
                nc.gpsimd.collective_compute(
                    "AllReduce", ADD, replica_groups=PAIRS,
                    ins=[og_sp[:].opt()], outs=[og_sum[:].opt()])

                # each core outputs its dm-half (fwd cols 0:512, bwd
                # 512:1024), int8-quantized with a per-time-row f32
                # scale (scale includes the 0.5 fwd/bwd average factor)
                for tb in range(8):
                    r0, r1 = tb * 128, (tb + 1) * 128
                    stp = pg.tile([128, 512], f16, tag="stp", bufs=2)
                    nc.sync.dma_start(stp[:], og_sum[r0:r1, 0:512])
                    sbt = pg.tile([128, 512], f16, tag="sbt", bufs=2)
                    nc.sync.dma_start(sbt[:], og_sum[r0:r1, 512:1024])
                    dfo = pg.tile([128, 512], f32, tag="dfo", bufs=2)
                    nc.vector.tensor_tensor(
                        out=dfo[:], in0=stp[:], in1=sbt[:], op=SUB)
                    sfo = pg.tile([128, 512], f32, tag="sfo", bufs=2)
                    nc.vector.tensor_scalar(
                        out=sfo[:], in0=dfo[:], scalar1=sel[:, 0:1],
                        scalar2=None, op0=MUL)
                    oh = pg.tile([128, 512], f32, tag="oh", bufs=2)
                    nc.vector.tensor_tensor(
                        out=oh[:], in0=sbt[:], in1=sfo[:], op=ADD)
                    rmax = pg.tile([128, 1], f32, tag="rmax", bufs=2)
                    nc.vector.tensor_reduce(
                        out=rmax[:], in_=oh[:], axis=AX.X, op=MAXO,
                        apply_absolute_value=True)
                    srow = pg.tile([128, 1], f32, tag="srow", bufs=2)
                    nc.vector.tensor_scalar(
                        out=srow[:], in0=rmax[:], scalar1=1e-30,
                        scalar2=1.0 / 127, op0=MAXO, op1=MUL)
                    rinv = pg.tile([128, 1], f32, tag="rinv", bufs=2)
                    nc.vector.reciprocal(rinv[:], srow[:])
                    sdl = pg.tile([128, 1], f32, tag="sdl", bufs=2)
                    nc.vector.tensor_scalar(
                        out=sdl[:], in0=srow[:], scalar1=0.5,
                        scalar2=None, op0=MUL)
                    q8 = pg.tile([128, 512], i8, tag="q8", bufs=2)
                    nc.scalar.activation(q8[:], oh[:], AF.Copy,
                                         scale=rinv[:, 0:1])
                    nc.sync.dma_start(out_d[r0:r1, 0:512],
                                      q8[:].bitcast(u8))
                    nc.sync.dma_start(out_d[r0:r1, 512:516],
                                      sdl[:].bitcast(u8))

    nc.compile()
    return nc


def _install_cc_cache():
    """Content-keyed disk cache around the neuronx compiler hook.

    The bass_exec compile path (walrus) takes ~60 s for this program and
    has no persistent cache of its own; the emitted BIR (and hence the
    HLO carrying it) is byte-deterministic, so a sha256-of-HLO keyed
    cache makes every process after the first skip the compile.
    """
    import os
    try:
        import libneuronxla
    except ImportError:
        return
    if getattr(libneuronxla, "_bass_cc_disk_cache", False):
        return
    inner = libneuronxla.neuronx_cc
    cache_dir = os.environ.get(
        "NEURON_COMPILE_CACHE_URL",
        os.path.join(os.path.expanduser("~"), ".neuron-compile-cache"))
    try:
        os.makedirs(cache_dir, exist_ok=True)
    except OSError:
        libneuronxla._bass_cc_disk_cache = True
        return

    def cached(code, code_format, platform_version, file_prefix,
               *a, **kw):
        c = code if isinstance(code, (bytes, bytearray)) else \
            str(code).encode()
        key = hashlib.sha256(
            c + b"|" + str(platform_version).encode()).hexdigest()
        path = os.path.join(cache_dir, f"bassneff-{key}.hlo")
        try:
            with open(path, "rb") as f:
                return 0, f.read()
        except OSError:
            pass
        r = inner(code, code_format, platform_version, file_prefix,
                  *a, **kw)
        try:
            err, blob = r
            if err == 0 and isinstance(blob, (bytes, bytearray)) and blob:
                tmp = f"{path}.tmp.{os.getpid()}"
                with open(tmp, "wb") as f:
                    f.write(blob)
                os.replace(tmp, path)
        except Exception:
            pass
        return r

    libneuronxla.neuronx_cc = cached
    libneuronxla._bass_cc_disk_cache = True


def _make_runner(nc):
    """Jit the SPMD dispatch once; repeat calls hit the C++ fast path.

    Mirrors what bass_utils.run_bass_kernel_spmd does under axon
    (bass2jax.run_bass_via_pjrt), minus the per-call re-jit and the
    donated zero output buffers (the kernel writes every output
    element, so uninitialized outputs are fine).
    """
    import jax
    import numpy as np
    from jax.sharding import Mesh, PartitionSpec
    from jax.experimental.shard_map import shard_map
    from concourse import mybir
    from concourse.bass2jax import (_bass_exec_p, install_neuronx_cc_hook,
                                    partition_id_tensor)

    install_neuronx_cc_hook()
    _install_cc_cache()
    partition_name = (nc.partition_id_tensor.name
                      if nc.partition_id_tensor else None)
    in_names, out_names, out_avals = [], [], []
    for alloc in nc.m.functions[0].allocations:
        if not isinstance(alloc, mybir.MemoryLocationSet):
            continue
        name = alloc.memorylocations[0].name
        if alloc.kind == "ExternalInput":
            if name != partition_name:
                in_names.append(name)
        elif alloc.kind == "ExternalOutput":
            out_names.append(name)
            out_avals.append(jax.core.ShapedArray(
                tuple(alloc.tensor_shape), mybir.dt.np(alloc.dtype)))
    in_names_all = list(in_names)
    if partition_name is not None:
        in_names_all.append(partition_name)

    def _body(*args):
        operands = list(args)
        if partition_name is not None:
            operands.append(partition_id_tensor())
        return tuple(_bass_exec_p.bind(
            *operands, out_avals=tuple(out_avals),
            in_names=tuple(in_names_all), out_names=tuple(out_names),
            lowering_input_output_aliases=(),
            sim_require_finite=True, sim_require_nnan=True, nc=nc))

    devices = jax.devices()[:NCORE]
    mesh = Mesh(np.asarray(devices), ("core",))
    n_in = len(in_names)
    sharded = jax.jit(shard_map(
        _body, mesh=mesh, in_specs=(PartitionSpec("core"),) * n_in,
        out_specs=(PartitionSpec("core"),) * len(out_names),
        check_rep=False))

    def run(per_name_globals):
        args = [per_name_globals[name] for name in in_names]
        outs = sharded(*args)
        return {name: outs[i] for i, name in enumerate(out_names)}

    return run


_NF = NA * 128
_BUF = None
_MESH = None


def _mesh():
    global _MESH
    if _MESH is None:
        import jax
        from jax.sharding import Mesh, PartitionSpec, NamedSharding
        devs = jax.devices()[:NCORE]
        m = Mesh(np.asarray(devs), ("core",))
        _MESH = (m, devs, NamedSharding(m, PartitionSpec("core")))
    return _MESH


_NW = _NF + 128


def _bufs():
    global _BUF
    if _BUF is None:
        _BUF = dict(
            gcmb=np.zeros((NCORE, 513, _NW), np.uint8),
            u16=np.empty((512, DQ), np.uint16),
            f32=np.empty((512, _NF), np.float32),
            y=[np.zeros((B, Lq, DM), np.float32) for _ in range(2)],
            yi=0,
        )
    return _BUF


def kernel(**inputs) -> np.ndarray:
    global _RUN, _WFP
    bufs = _bufs()
    qf = np.asarray(inputs["query"], np.float32)
    cf_ = np.asarray(inputs["context"], np.float32)
    # ctx: 8-bit code u = round(x*s_c)+128; q: 9-bit u = round(x*s_q)+256.
    # Scales only need to agree within a core pair (= per batch), so
    # they are computed lazily per batch — the first put hits the wire
    # after one batch's amax pass, not a global one. All host passes run
    # on contiguous views (single CPU core).
    gcmb = bufs["gcmb"]
    u16, ftmp = bufs["u16"], bufs["f32"]
    s_cb, s_qb = np.empty(B), np.empty(B)
    # device_put per core as soon as it is packed: puts are async, so
    # the tunnel upload streams while the host packs the next core
    import jax
    devs, shards = _mesh()[1], []
    for core in range(NCORE):
        b, hf = core % 4, core // 4
        if hf == 0:
            s_cb[b] = 127.0 / max(np.abs(cf_[b]).max(), 1e-30)
            s_qb[b] = 255.0 / max(np.abs(qf[b]).max(), 1e-30)
        s_c, s_q = s_cb[b], s_qb[b]
        hs = slice(hf * 512, (hf + 1) * 512)
        g = gcmb[core]
        np.multiply(cf_[b, hs], s_c, out=ftmp[:, :DC])
        np.add(ftmp[:, :DC], 128.5, out=g[:512, :DC], casting="unsafe")
        np.multiply(qf[b, hs], s_q, out=ftmp[:, DC:])
        np.add(ftmp[:, DC:], 256.5, out=u16, casting="unsafe")
        np.bitwise_and(u16, 255, out=g[:512, DC:_NF], casting="unsafe")
        hb = (u16 >> 8).astype(np.uint8).reshape(512, 128, 8)
        g[:512, _NF:] = np.packbits(hb, axis=2, bitorder="little")[:, :, 0]
        mrow = np.array([1.0 - hf, 1.0 / s_c, 1.0 / s_q, 0.0], np.float32)
        g[512, :16] = mrow.view(np.uint8)
        shards.append(jax.device_put(g, devs[core]))

    # fingerprint the weights while the upload streams
    fp = _fingerprint(inputs)
    if _RUN is None or fp != _WFP:
        _RUN = _make_runner(_build(_prep_weights(inputs)))
        _WFP = fp

    ga = jax.make_array_from_single_device_arrays(
        (NCORE * 513, _NW), _mesh()[2], shards)
    res = _RUN({"acmb": ga})
    o = np.asarray(res["out"]).reshape(NCORE, Lq, DM // 2 + 4)
    q8 = o[:, :, :DM // 2].view(np.int8)
    sc = np.ascontiguousarray(o[:, :, DM // 2:]).view(np.float32)
    bufs["yi"] ^= 1
    y = bufs["y"][bufs["yi"]]
    for b in range(B):
        np.multiply(q8[b], sc[b], out=y[b, :, :DM // 2])
        np.multiply(q8[4 + b], sc[4 + b], out=y[b, :, DM // 2:])
    return y

